# revision 2
# baseline (speedup 1.0000x reference)
"""Trainium2 Bass kernel: CLUTRR-style GNN message passing (nn_CLUTRRV4). v3.

Data-parallel across 8 NeuronCores. Per core, 256 samples are packed
4-per-group (4 x 32 entity slots = 128 partitions). Key differences vs v2
baseline:
  * All one-hot gather/scatter/rel matrices are precomputed on the HOST and
    DMA'd once (they are constant across the 8 message-passing steps) --
    removes ~800us of DVE is_equal work per core.
  * Edge sparsity packing: samples are re-assigned (globally across cores,
    then into per-core groups) so each thin group's VALID edges fit in 128
    columns (vs 256 dense). 62 thin groups (cap 128) + 2 fat groups (cap
    256, absorbing the largest samples) per core. Output is un-permuted on
    the host.
  * S is kept in f16 only (no f32 master, no shadow refresh).
  * msg_b2 * indegree is folded in as a K=1 matmul per pair (was per group).
  * PSUM->SBUF copies balanced across Vector and Scalar engines.
"""
import sys
import numpy as np

if "/opt/trn_rl_repo" not in sys.path:
    sys.path.append("/opt/trn_rl_repo")

N_ENT, N_REL, D, E = 32, 20, 128, 64
N_STEPS = 8
N_CORES = 8
P = 128
GRP = 4  # samples per group


def _patch_ldw_opt():
    import os
    if os.environ.get("BASS_LDW_OPT") != "1":
        return
    from concourse import bass_utils as bu
    if getattr(bu, "_ldw_opt_patched", False):
        return
    orig = bu.run_command

    def run_command_ldw(cmd, *a, **kw):
        if isinstance(cmd, list):
            cmd = [c.replace("--enable-ldw-opt=false", "--enable-ldw-opt=true")
                   if isinstance(c, str) else c for c in cmd]
        return orig(cmd, *a, **kw)

    bu.run_command = run_command_ldw
    bu._ldw_opt_patched = True


def _caps_for(b_core):
    """Per-group edge capacities. 256 samples -> 62 thin (128) + 2 fat (256).
    Other sizes -> all-fat fallback (always packable: 4*63 <= 256)."""
    G = b_core // GRP
    if b_core == 256:
        return [128] * 62 + [256, 256]
    return [256] * G


def _build_nc(b_core, n_steps, debug_dump=False):
    from concourse import bacc, mybir
    from concourse.tile import TileContext
    from concourse.masks import make_identity

    f32 = mybir.dt.float32
    f16 = mybir.dt.float16
    AF = mybir.ActivationFunctionType
    OP = mybir.AluOpType

    caps = _caps_for(b_core)
    G = b_core // GRP
    NPAIR = G // 2
    n_thin = sum(1 for c in caps if c == 128)
    n_reltile = (n_thin + 3) // 4
    nfat = G - n_thin

    nc = bacc.Bacc()

    def din(name, shape, dtype=f16):
        return nc.declare_dram_parameter(name, list(shape), dtype, isOutput=False)

    # bundled per-pair one-hot block:
    # [rel-g0 | rel-g1 | ohs-g0 | ohs-g1 | oht-g0 | oht-g1 | ohe-g0 | ohe-g1]
    # thin pair: 2*256 + 6*128 = 1280 cols; fat pair: 2*512 + 6*256 = 2560
    n_thinp = n_thin // 2
    n_fatp = nfat // 2
    d_s0 = din("s0", (P, 32))
    d_bund_t = din("bund_t", (n_thinp, P, 1280)) if n_thinp else None
    d_bund_f = din("bund_f", (n_fatp, P, 2560)) if n_fatp else None
    d_indb = din("indb", (NPAIR // 8, P, 8 * 256))
    d_qoh = din("qoh", (P, G * 8))
    d_rt2 = din("rt2", (P, 128))
    d_b2m = din("b2m", (P, 1), f32)
    d_w1ac = din("w1ac", (P, 512))
    d_w2m = din("w2m", (P, 256))
    d_w1u = din("w1u", (P, 512))
    d_w2u = din("w2u", (P, 256))
    d_b1u = din("b1u", (P, 2), f32)
    d_b2u = din("b2u", (P, 1), f32)
    d_cw1 = din("cw1", (P, 256))
    d_cb1 = din("cb1", (P, 1), f32)
    d_cw2 = din("cw2", (P, 128))
    d_cb2 = din("cb2", (20, 1), f32)
    d_out = nc.declare_dram_parameter("out", [20, b_core], f32, isOutput=True)
    d_sdump = (nc.declare_dram_parameter("sdump", [P, (b_core // GRP) * P], f16,
                                         isOutput=True) if debug_dump else None)

    with TileContext(nc) as tc:
        with (
            tc.tile_pool(name="c", bufs=1) as cp,
            tc.tile_pool(name="w", bufs=4) as wp,
            tc.tile_pool(name="pA", bufs=2, space="PSUM") as pA,
            tc.tile_pool(name="pH1", bufs=2, space="PSUM") as pH1,
            tc.tile_pool(name="pM", bufs=1, space="PSUM") as pM,
            tc.tile_pool(name="pG", bufs=1, space="PSUM") as pG,
            tc.tile_pool(name="pH3", bufs=1, space="PSUM") as pH3,
            tc.tile_pool(name="pS", bufs=1, space="PSUM") as pS,
        ):
            def cload(name, shape, dram, dtype=f16):
                t = cp.tile(list(shape), dtype, tag=name, name=name)
                nc.sync.dma_start(t[:], dram[:])
                return t

            w1ac = cload("w1ac", (P, 512), d_w1ac)
            w2m = cload("w2m", (P, 256), d_w2m)
            w1u = cload("w1u", (P, 512), d_w1u)
            w2u = cload("w2u", (P, 256), d_w2u)
            rt2 = cload("rt2", (P, 128), d_rt2)
            b2m = cload("b2m", (P, 1), d_b2m, f32)
            b1u = cload("b1u", (P, 2), d_b1u, f32)
            b2u = cload("b2u", (P, 1), d_b2u, f32)
            cw1 = cload("cw1", (P, 256), d_cw1)
            cb1 = cload("cb1", (P, 1), d_cb1, f32)
            cw2 = cload("cw2", (P, 128), d_cw2)
            cb2 = cload("cb2", (20, 1), d_cb2, f32)

            outsb = cp.tile([20, b_core], f32, tag="outsb", name="outsb")

            s0seed = cp.tile([P, 32], f16, tag="s0seed", name="s0seed")
            nc.sync.dma_start(s0seed[:], d_s0[:])

            S = [None] * NPAIR
            Spair2 = [None] * (NPAIR // 2)
            OHS, OHT, OHE, QOH, REL = ([None] * G for _ in range(5))
            INDB = [None] * NPAIR
            thin_p = 0
            fat_p = 0
            for p in range(NPAIR):
                if p % 2 == 0:
                    t = cp.tile([P, 512], f16, tag=f"S{p // 2}", name=f"S{p // 2}")
                    nc.vector.tensor_copy(
                        t[:].rearrange("p (r c) -> p r c", c=32),
                        s0seed[:, None, :].to_broadcast([P, 16, 32]))
                    Spair2[p // 2] = t[:]
                S[p] = Spair2[p // 2][:, (p % 2) * 256:(p % 2 + 1) * 256]
                if p % 8 == 0:
                    t = cp.tile([P, 8 * 256], f16, tag=f"ib{p // 8}",
                                name=f"ib{p // 8}")
                    nc.sync.dma_start(t[:], d_indb[p // 8])
                    ibt = t
                if p % 2 == 0:
                    INDB[p] = ibt[:, (p % 8) * 256:(p % 8 + 2) * 256]  # 512 wide
                fat = caps[2 * p] == 256
                w = 2560 if fat else 1280
                bund = cp.tile([P, w], f16, tag=f"bk{p}", name=f"bk{p}")
                dma_eng = nc.sync if p % 2 == 0 else nc.scalar
                if fat:
                    dma_eng.dma_start(bund[:], d_bund_f[fat_p])
                    fat_p += 1
                else:
                    dma_eng.dma_start(bund[:], d_bund_t[thin_p])
                    thin_p += 1
                cw = w // 10  # 128 thin, 256 fat (per-group one-hot width)
                for gi in (0, 1):
                    g = 2 * p + gi
                    REL[g] = bund[:, gi * 2 * cw:(gi + 1) * 2 * cw]
                    OHS[g] = bund[:, 4 * cw + gi * cw:4 * cw + (gi + 1) * cw]
                    OHT[g] = bund[:, 6 * cw + gi * cw:6 * cw + (gi + 1) * cw]
                    OHE[g] = bund[:, 8 * cw + gi * cw:8 * cw + (gi + 1) * cw]
            qoht = cp.tile([P, G * 8], f16, tag="qoh", name="qoh")
            nc.sync.dma_start(qoht[:], d_qoh[:])
            for g in range(G):
                QOH[g] = qoht[:, g * 8:(g + 1) * 8]

            stsbig = cp.tile([P, G * P], f16, tag="stsbig", name="stsbig")
            mm = nc.tensor.matmul

            for t_step in range(n_steps):
                for pb in range(NPAIR // 2):
                  for pi in (0, 1):
                    p = 2 * pb + pi
                    fat = caps[2 * p] == 256
                    aggt = pG.tile([P, 256], f32, tag="agg", name="agg")
                    agg = aggt[:]
                    # A = [S@W1a | S@W1c] slot-major, one tile per group
                    asb = wp.tile([P, 1024], f16, tag="asb", name="asb")
                    for gi in (0, 1):
                        aps = pA.tile([P, 512], f32, tag="aps", name="aps")
                        mm(aps[:], lhsT=S[p][:, gi * P:(gi + 1) * P], rhs=w1ac[:],
                           start=True, stop=True)
                        nc.vector.tensor_copy(asb[:, gi * 512:(gi + 1) * 512], aps[:])

                    if not fat:
                        # h1 for both groups in one 1-bank tile; per group:
                        # one rel MM (band trick: rows 0:20 F0 table, 32:52 F1)
                        # + 4 gather MMs. Each group's region closes before the
                        # next group's start (start clears bank has_written).
                        h1 = pH1.tile([P, 512], f32, tag="h1", name="h1")
                        for gi in (0, 1):
                            g = 2 * p + gi
                            base = gi * 256
                            mm(h1[:, base:base + 256], lhsT=rt2[:],
                               rhs=REL[g], start=True, stop=False)
                            for F in range(2):
                                o = h1[:, base + F * P:base + (F + 1) * P]
                                mm(o, lhsT=asb[:, gi * 512 + F * P:gi * 512 + (F + 1) * P],
                                   rhs=OHS[g], start=False, stop=False)
                                mm(o, lhsT=asb[:, gi * 512 + 256 + F * P:gi * 512 + 256 + (F + 1) * P],
                                   rhs=OHT[g], start=False, stop=True)
                        h1g = wp.tile([P, 512], f16, tag="h1g", name="h1g")
                        nc.scalar.activation(h1g[:], h1[:], AF.Gelu)
                        # msg layer 2, edge-major (pos, D), both groups
                        msg = pM.tile([P, 256], f32, tag="msg", name="msg")
                        for gi in (0, 1):
                            o = msg[:, gi * P:(gi + 1) * P]
                            for F in range(2):
                                mm(o, lhsT=h1g[:, gi * 256 + F * P:gi * 256 + (F + 1) * P],
                                   rhs=w2m[:, F * P:(F + 1) * P],
                                   start=(F == 0), stop=(F == 1))
                        msb = wp.tile([P, 256], f16, tag="msb", name="msb")
                        nc.scalar.copy(msb[:], msg[:])
                        for gi in (0, 1):
                            mm(agg[:, gi * P:(gi + 1) * P],
                               lhsT=msb[:, gi * P:(gi + 1) * P], rhs=OHE[2 * p + gi],
                               start=True, stop=True)
                    else:
                        for gi in (0, 1):
                            g = 2 * p + gi
                            h1 = pH1.tile([P, 512], f32, tag="h1", name="h1")
                            mm(h1[:], lhsT=rt2[:], rhs=REL[g][:],
                               start=True, stop=False)
                            for F in range(2):
                                o = h1[:, F * 256:(F + 1) * 256]
                                mm(o, lhsT=asb[:, gi * 512 + F * P:gi * 512 + (F + 1) * P],
                                   rhs=OHS[g], start=False, stop=False)
                                mm(o, lhsT=asb[:, gi * 512 + 256 + F * P:gi * 512 + 256 + (F + 1) * P],
                                   rhs=OHT[g], start=False, stop=True)
                            h1g = wp.tile([P, 512], f16, tag="h1g", name="h1g")
                            nc.scalar.activation(h1g[:], h1[:], AF.Gelu)
                            msg = pM.tile([P, 256], f32, tag="msg", name="msg")
                            for ec in range(2):
                                o = msg[:, ec * P:(ec + 1) * P]
                                for F in range(2):
                                    mm(o, lhsT=h1g[:, F * 256 + ec * P:F * 256 + (ec + 1) * P],
                                       rhs=w2m[:, F * P:(F + 1) * P],
                                       start=(F == 0), stop=(F == 1))
                            msb = wp.tile([P, 256], f16, tag="msb", name="msb")
                            nc.scalar.copy(msb[:], msg[:])
                            o = agg[:, gi * P:(gi + 1) * P]
                            for ec in range(2):
                                mm(o, lhsT=msb[:, ec * P:(ec + 1) * P],
                                   rhs=OHE[g][:, ec * P:(ec + 1) * P],
                                   start=(ec == 0), stop=(ec == 1))
                    # gsb = agg + msg_b2 * indegree (DVE)
                    gsbt = wp.tile([P, 256], f16, tag="gsb", name="gsb")
                    nc.vector.scalar_tensor_tensor(
                        out=gsbt[:], in0=INDB[2 * pb][:, pi * 256:(pi + 1) * 256],
                        scalar=b2m[:, 0:1],
                        in1=agg, op0=OP.mult, op1=OP.add)
                    gsb = gsbt[:]
                    # update MLP over the pair (256 slot cols)
                    h3 = pH3.tile([P, 512], f32, tag="h3", name="h3")
                    for mc in range(2):
                        o = h3[:, mc * 256:(mc + 1) * 256]
                        mm(o, lhsT=w1u[:, mc * P:(mc + 1) * P], rhs=S[p],
                           start=True, stop=False)
                        mm(o, lhsT=w1u[:, 256 + mc * P:256 + (mc + 1) * P], rhs=gsb,
                           start=False, stop=True)
                    h3g = wp.tile([P, 512], f16, tag="h3g", name="h3g")
                    for mc in range(2):
                        nc.scalar.activation(
                            h3g[:, mc * 256:(mc + 1) * 256],
                            h3[:, mc * 256:(mc + 1) * 256], AF.Gelu,
                            bias=b1u[:, mc:mc + 1])
                    sn = pS.tile([P, 256], f32, tag="sn", name="sn")
                    for kc in range(2):
                        mm(sn[:], lhsT=w2u[:, kc * P:(kc + 1) * P],
                           rhs=h3g[:, kc * 256:(kc + 1) * 256],
                           start=(kc == 0), stop=(kc == 1))
                    # S += sn + b2u, f16 in place
                    nc.vector.scalar_tensor_tensor(
                        out=S[p], in0=sn[:], scalar=b2u[:, 0:1], in1=S[p],
                        op0=OP.add, op1=OP.add)
                    if t_step == n_steps - 1:
                        for gi in (0, 1):
                            g = 2 * p + gi
                            nc.sync.dma_start_transpose(
                                stsbig[:, g * P:(g + 1) * P],
                                S[p][:, gi * P:(gi + 1) * P])

            if debug_dump:
                for p in range(NPAIR):
                    nc.sync.dma_start(d_sdump[:, p * 256:(p + 1) * 256], S[p])
            # classifier head (f16 matmuls)
            nbatch = (G + 15) // 16
            for bq in range(nbatch):
                jn = min(16, G - bq * 16)
                qps = pH1.tile([P, P], f32, tag="h1", name="qps")
                for j in range(jn):
                    g = bq * 16 + j
                    mm(qps[:, j * 8:(j + 1) * 8], lhsT=stsbig[:, g * P:(g + 1) * P],
                       rhs=QOH[g], start=True, stop=True)
                qcat = wp.tile([P, P], f16, tag="qcat", name="qcat")
                nc.vector.tensor_copy(qcat[:, 0:jn * 8], qps[:, 0:jn * 8])
                qv = qcat[:, 0:jn * 8].rearrange("p (g t f) -> p g t f", t=2, f=4)
                ncols = jn * 4
                hps = pM.tile([P, ncols], f32, tag="msg", name="hps",
                              padded_shape=[P, 256])
                mm(hps[:], lhsT=cw1[:, 0:P], rhs=qv[:, :, 0, :], start=True, stop=False)
                mm(hps[:], lhsT=cw1[:, P:256], rhs=qv[:, :, 1, :], start=False, stop=True)
                hg = wp.tile([P, ncols], f16, tag="hg", name="hg",
                             padded_shape=[P, 64])
                nc.scalar.activation(hg[:], hps[:], AF.Gelu, bias=cb1[:, 0:1])
                ops_ = pG.tile([P, ncols], f32, tag="agg", name="ops",
                               padded_shape=[P, 256])
                mm(ops_[:], lhsT=cw2[:], rhs=hg[:], start=True, stop=True)
                nc.scalar.activation(
                    outsb[:, bq * 64:bq * 64 + ncols], ops_[0:20, :], AF.Identity,
                    bias=cb2[:, 0:1])
            nc.sync.dma_start(d_out[:], outsb[:])

    nc.finalize()
    return nc


def _pack_samples(ne, b_core):
    """Global balanced assignment: returns per-core list of
    (group, j) -> original sample index arrays, shape (n_cores, G, 4)."""
    B = ne.shape[0]
    n_cores = B // b_core
    G = b_core // GRP
    caps = _caps_for(b_core)
    n_thin = sum(1 for c in caps if c == 128)
    nfat = G - n_thin

    order = np.argsort(-ne, kind="stable")
    core_sum = np.zeros(n_cores, dtype=np.int64)
    core_cnt = np.zeros(n_cores, dtype=np.int64)
    core_members = [[] for _ in range(n_cores)]
    for s in order:
        avail = np.where(core_cnt < b_core)[0]
        c = avail[np.argmin(core_sum[avail])]
        core_members[c].append(s)
        core_sum[c] += ne[s]
        core_cnt[c] += 1

    layout = np.zeros((n_cores, G, GRP), dtype=np.int64)
    ok = True
    for c in range(n_cores):
        mem = np.asarray(core_members[c])
        o = np.argsort(-ne[mem], kind="stable")
        mem = mem[o]
        fat = mem[:4 * nfat]
        rest = mem[4 * nfat:]
        # fat groups occupy the END of the group list (indices n_thin..G-1)
        for fg in range(nfat):
            layout[c, n_thin + fg] = fat[fg * 4:(fg + 1) * 4]
        nr = ne[rest]
        sums = np.zeros(n_thin, dtype=np.int64)
        cnt = np.zeros(n_thin, dtype=np.int64)
        for k in range(rest.shape[0]):
            avail = np.where(cnt < GRP)[0]
            gsel = avail[np.argmin(sums[avail])]
            layout[c, gsel, cnt[gsel]] = rest[k]
            sums[gsel] += nr[k]
            cnt[gsel] += 1
        if n_thin and sums.max() > 128:
            ok = False
    return layout, ok


def _host_prep(inputs, b_core):
    f = np.float32
    h = np.float16
    src_a = np.asarray(inputs["edge_src"]).astype(np.int64)
    tgt_a = np.asarray(inputs["edge_tgt"]).astype(np.int64)
    rel_a = np.asarray(inputs["edge_rel"]).astype(np.int64)
    ne = np.asarray(inputs["n_edges"]).astype(np.int64)
    qs_a = np.asarray(inputs["query_src"]).astype(np.int64)
    qt_a = np.asarray(inputs["query_tgt"]).astype(np.int64)
    B = src_a.shape[0]
    n_cores = B // b_core
    G = b_core // GRP
    NPAIR = G // 2
    caps = _caps_for(b_core)
    n_thin = sum(1 for c in caps if c == 128)
    nfat = G - n_thin
    n_reltile = (n_thin + 3) // 4

    layout, ok = _pack_samples(ne, b_core)
    if not ok:
        # fall back: no thin groups (callers rebuild nc keyed by caps anyway)
        raise RuntimeError("edge packing overflow; unsupported input")

    # shared params
    ee = np.asarray(inputs["entity_embed"], f)
    w1 = np.asarray(inputs["msg_W1"], f)
    reltab = np.asarray(inputs["rel_embed"], f) @ w1[128:256] + np.asarray(inputs["msg_b1"], f)
    # band layout: rows 0:20 hold the F0 half of the rel table, rows 32:52 the
    # F1 half; the rel one-hot rhs selects the right band per column range
    rt2 = np.zeros((P, 128), f)
    rt2[0:20] = reltab[:, 0:128]
    rt2[32:52] = reltab[:, 128:256]
    w2m_ = np.asarray(inputs["msg_W2"], f)
    w1u_ = np.asarray(inputs["upd_W1"], f)
    w2u_ = np.asarray(inputs["upd_W2"], f)
    cw1_ = np.asarray(inputs["cls_W1"], f)
    cw2p = np.zeros((P, 128), f)
    cw2p[:, 0:20] = np.asarray(inputs["cls_W2"], f)
    shared = {
        "s0": np.ascontiguousarray(ee.T.astype(h)),
        "rt2": rt2.astype(h),
        "b2m": np.asarray(inputs["msg_b2"], f).reshape(128, 1).copy(),
        "w1ac": np.concatenate([w1[0:128], w1[256:384]], axis=1).astype(h),
        "w2m": np.concatenate([w2m_[0:128], w2m_[128:256]], axis=1).astype(h),
        "w1u": np.concatenate(
            [w1u_[0:128, 0:128], w1u_[0:128, 128:256],
             w1u_[128:256, 0:128], w1u_[128:256, 128:256]], axis=1).astype(h),
        "w2u": np.concatenate([w2u_[0:128], w2u_[128:256]], axis=1).astype(h),
        "b1u": np.asarray(inputs["upd_b1"], f).reshape(2, 128).T.copy(),
        "b2u": np.asarray(inputs["upd_b2"], f).reshape(128, 1).copy(),
        "cw1": np.concatenate([cw1_[0:128], cw1_[128:256]], axis=1).astype(h),
        "cb1": np.asarray(inputs["cls_b1"], f).reshape(128, 1).copy(),
        "cw2": cw2p.astype(h),
        "cb2": np.asarray(inputs["cls_b2"], f).reshape(20, 1).copy(),
    }

    n_thinp = n_thin // 2
    n_fatp = nfat // 2
    in_maps = []
    perm = np.zeros(B, dtype=np.int64)  # out row (c*b_core + g*4 + j) -> sample
    for c in range(n_cores):
        bund_t = np.zeros((max(1, n_thinp), P, 1280), h)
        bund_f = np.zeros((max(1, n_fatp), P, 2560), h)
        ind = np.zeros((NPAIR, 256), np.float32)
        qoh = np.zeros((P, G * 8), h)

        thin_p = 0
        fat_p = 0
        for p in range(NPAIR):
            fatpair = caps[2 * p] == 256
            if fatpair:
                bund = bund_f[fat_p]
                fat_p += 1
            else:
                bund = bund_t[thin_p]
                thin_p += 1
            cap = caps[2 * p]
            cw = cap
            for gi in (0, 1):
                g = 2 * p + gi
                rtile = bund[:, gi * 2 * cw:(gi + 1) * 2 * cw]
                ohs = bund[:, 4 * cw + gi * cw:4 * cw + (gi + 1) * cw]
                oht = bund[:, 6 * cw + gi * cw:6 * cw + (gi + 1) * cw]
                ohe = bund[:, 8 * cw + gi * cw:8 * cw + (gi + 1) * cw]
                pos = 0
                for j in range(GRP):
                    s = layout[c, g, j]
                    perm[c * b_core + g * GRP + j] = s
                    nedge = ne[s]
                    for e in range(nedge):
                        ohs[j * 32 + src_a[s, e], pos] = 1
                        oht[j * 32 + tgt_a[s, e], pos] = 1
                        ec, pr = divmod(pos, 128)
                        ohe[pr, ec * 128 + j * 32 + tgt_a[s, e]] = 1
                        rtile[rel_a[s, e], pos] = 1
                        rtile[32 + rel_a[s, e], cap + pos] = 1
                        ind[p, gi * 128 + j * 32 + tgt_a[s, e]] += 1
                        pos += 1
                    assert pos <= cap
                    qoh[j * 32 + qs_a[s], g * 8 + j] = 1
                    qoh[j * 32 + qt_a[s], g * 8 + 4 + j] = 1

        indb = np.ascontiguousarray(
            np.broadcast_to(ind[:, None, :], (NPAIR, P, 256))
            .reshape(NPAIR // 8, 8, P, 256).transpose(0, 2, 1, 3)
            .reshape(NPAIR // 8, P, 8 * 256)).astype(h)
        m = dict(shared)
        if n_thinp:
            m["bund_t"] = bund_t
        if n_fatp:
            m["bund_f"] = bund_f
        m.update({"indb": indb, "qoh": qoh})
        in_maps.append(m)
    return in_maps, perm


_CACHE = {}


def _make_in_maps(inputs, b_core):
    return _host_prep(inputs, b_core)[0]


def kernel(**inputs):
    b = np.asarray(inputs["edge_src"]).shape[0]
    b_core = b // N_CORES
    _patch_ldw_opt()
    if b_core not in _CACHE:
        _CACHE[b_core] = _build_nc(b_core, N_STEPS)
    nc = _CACHE[b_core]

    in_maps, perm = _host_prep(inputs, b_core)

    from concourse.bass_utils import run_bass_kernel_spmd
    res = run_bass_kernel_spmd(nc, in_maps, core_ids=list(range(N_CORES)))
    packed = np.concatenate([r["out"].T for r in res.results], axis=0)
    out = np.zeros_like(packed)
    out[perm] = packed
    return np.ascontiguousarray(out, dtype=np.float32)


# revision 3
# speedup vs baseline: 1.0049x; 1.0049x over previous
"""Trainium2 Bass kernel: CLUTRR-style GNN message passing (nn_CLUTRRV4). v3.

Data-parallel across 8 NeuronCores. Per core, 256 samples are packed
4-per-group (4 x 32 entity slots = 128 partitions). Key differences vs v2
baseline:
  * All one-hot gather/scatter/rel matrices are precomputed on the HOST and
    DMA'd once (they are constant across the 8 message-passing steps) --
    removes ~800us of DVE is_equal work per core.
  * Edge sparsity packing: samples are re-assigned (globally across cores,
    then into per-core groups) so each thin group's VALID edges fit in 128
    columns (vs 256 dense). 62 thin groups (cap 128) + 2 fat groups (cap
    256, absorbing the largest samples) per core. Output is un-permuted on
    the host.
  * S is kept in f16 only (no f32 master, no shadow refresh).
  * msg_b2 * indegree is folded in as a K=1 matmul per pair (was per group).
  * PSUM->SBUF copies balanced across Vector and Scalar engines.
"""
import sys
import numpy as np

if "/opt/trn_rl_repo" not in sys.path:
    sys.path.append("/opt/trn_rl_repo")

N_ENT, N_REL, D, E = 32, 20, 128, 64
N_STEPS = 8
N_CORES = 8
P = 128
GRP = 4  # samples per group


def _patch_ldw_opt():
    import os
    if os.environ.get("BASS_LDW_OPT") != "1":
        return
    from concourse import bass_utils as bu
    if getattr(bu, "_ldw_opt_patched", False):
        return
    orig = bu.run_command

    def run_command_ldw(cmd, *a, **kw):
        if isinstance(cmd, list):
            cmd = [c.replace("--enable-ldw-opt=false", "--enable-ldw-opt=true")
                   if isinstance(c, str) else c for c in cmd]
        return orig(cmd, *a, **kw)

    bu.run_command = run_command_ldw
    bu._ldw_opt_patched = True


def _caps_for(b_core):
    """Per-group edge capacities. 256 samples -> 62 thin (128) + 2 fat (256).
    Other sizes -> all-fat fallback (always packable: 4*63 <= 256)."""
    G = b_core // GRP
    if b_core == 256:
        return [128] * 62 + [256, 256]
    return [256] * G


def _build_nc(b_core, n_steps, debug_dump=False):
    from concourse import bacc, mybir
    from concourse.tile import TileContext
    from concourse.masks import make_identity

    f32 = mybir.dt.float32
    f16 = mybir.dt.float16
    AF = mybir.ActivationFunctionType
    OP = mybir.AluOpType

    caps = _caps_for(b_core)
    G = b_core // GRP
    NPAIR = G // 2
    n_thin = sum(1 for c in caps if c == 128)
    n_reltile = (n_thin + 3) // 4
    nfat = G - n_thin

    nc = bacc.Bacc()

    def din(name, shape, dtype=f16):
        return nc.declare_dram_parameter(name, list(shape), dtype, isOutput=False)

    # bundled per-pair one-hot block:
    # [rel-g0 | rel-g1 | ohs-g0 | ohs-g1 | oht-g0 | oht-g1 | ohe-g0 | ohe-g1]
    # thin pair: 2*256 + 6*128 = 1280 cols; fat pair: 2*512 + 6*256 = 2560
    n_thinp = n_thin // 2
    n_fatp = nfat // 2
    d_s0 = din("s0", (P, 32))
    d_bund_t = din("bund_t", (n_thinp, P, 1280)) if n_thinp else None
    d_bund_f = din("bund_f", (n_fatp, P, 2560)) if n_fatp else None
    d_indb = din("indb", (NPAIR // 8, P, 8 * 256))
    d_qoh = din("qoh", (P, G * 8))
    d_rt2 = din("rt2", (P, 128))
    d_b2m = din("b2m", (P, 1), f32)
    d_w1ac = din("w1ac", (P, 512))
    d_w2m = din("w2m", (P, 256))
    d_w1u = din("w1u", (P, 512))
    d_w2u = din("w2u", (P, 256))
    d_b1u = din("b1u", (P, 2), f32)
    d_b2u = din("b2u", (P, 1), f32)
    d_cw1 = din("cw1", (P, 256))
    d_cb1 = din("cb1", (P, 1), f32)
    d_cw2 = din("cw2", (P, 128))
    d_cb2 = din("cb2", (20, 1), f32)
    d_out = nc.declare_dram_parameter("out", [20, b_core], f32, isOutput=True)
    d_sdump = (nc.declare_dram_parameter("sdump", [P, (b_core // GRP) * P], f16,
                                         isOutput=True) if debug_dump else None)

    with TileContext(nc) as tc:
        with (
            tc.tile_pool(name="c", bufs=1) as cp,
            tc.tile_pool(name="w", bufs=4) as wp,
            tc.tile_pool(name="pA", bufs=2, space="PSUM") as pA,
            tc.tile_pool(name="pH1", bufs=2, space="PSUM") as pH1,
            tc.tile_pool(name="pM", bufs=1, space="PSUM") as pM,
            tc.tile_pool(name="pG", bufs=1, space="PSUM") as pG,
            tc.tile_pool(name="pH3", bufs=1, space="PSUM") as pH3,
            tc.tile_pool(name="pS", bufs=1, space="PSUM") as pS,
        ):
            def cload(name, shape, dram, dtype=f16):
                t = cp.tile(list(shape), dtype, tag=name, name=name)
                nc.sync.dma_start(t[:], dram[:])
                return t

            S = [None] * NPAIR
            Spair2 = [None] * (NPAIR // 2)
            OHS, OHT, OHE, QOH, REL = ([None] * G for _ in range(5))
            INDB = [None] * NPAIR
            IBT = [None] * (NPAIR // 8)
            fat_flags = [caps[2 * p] == 256 for p in range(NPAIR)]
            thin_idx = np.cumsum([0] + [0 if f else 1 for f in fat_flags]).tolist()
            fat_idx = np.cumsum([0] + [1 if f else 0 for f in fat_flags]).tolist()

            s0seed = cp.tile([P, 32], f16, tag="s0seed", name="s0seed")
            nc.sync.dma_start(s0seed[:], d_s0[:])

            def load_pair(p):
                if p % 2 == 0:
                    t = cp.tile([P, 512], f16, tag=f"S{p // 2}", name=f"S{p // 2}")
                    nc.vector.tensor_copy(
                        t[:].rearrange("p (r c) -> p r c", c=32),
                        s0seed[:, None, :].to_broadcast([P, 16, 32]))
                    Spair2[p // 2] = t[:]
                S[p] = Spair2[p // 2][:, (p % 2) * 256:(p % 2 + 1) * 256]
                if p % 8 == 0:
                    t = cp.tile([P, 8 * 256], f16, tag=f"ib{p // 8}",
                                name=f"ib{p // 8}")
                    nc.sync.dma_start(t[:], d_indb[p // 8])
                    IBT[p // 8] = t
                if p % 2 == 0:
                    INDB[p] = IBT[p // 8][:, (p % 8) * 256:(p % 8 + 2) * 256]
                fat = fat_flags[p]
                w = 2560 if fat else 1280
                bund = cp.tile([P, w], f16, tag=f"bk{p}", name=f"bk{p}")
                dma_eng = nc.sync if p % 2 == 0 else nc.scalar
                if fat:
                    dma_eng.dma_start(bund[:], d_bund_f[fat_idx[p]])
                else:
                    dma_eng.dma_start(bund[:], d_bund_t[thin_idx[p]])
                cw = w // 10  # 128 thin, 256 fat (per-group one-hot width)
                for gi in (0, 1):
                    g = 2 * p + gi
                    REL[g] = bund[:, gi * 2 * cw:(gi + 1) * 2 * cw]
                    OHS[g] = bund[:, 4 * cw + gi * cw:4 * cw + (gi + 1) * cw]
                    OHT[g] = bund[:, 6 * cw + gi * cw:6 * cw + (gi + 1) * cw]
                    OHE[g] = bund[:, 8 * cw + gi * cw:8 * cw + (gi + 1) * cw]

            # step-0 critical-path weights + first pairs' data first
            w1ac = cload("w1ac", (P, 512), d_w1ac)
            rt2 = cload("rt2", (P, 128), d_rt2)
            load_pair(0)
            load_pair(1)
            w2m = cload("w2m", (P, 256), d_w2m)
            w1u = cload("w1u", (P, 512), d_w1u)
            w2u = cload("w2u", (P, 256), d_w2u)
            b2m = cload("b2m", (P, 1), d_b2m, f32)
            b1u = cload("b1u", (P, 2), d_b1u, f32)
            b2u = cload("b2u", (P, 1), d_b2u, f32)
            for p in range(2, NPAIR):
                load_pair(p)
            cw1 = cload("cw1", (P, 256), d_cw1)
            cb1 = cload("cb1", (P, 1), d_cb1, f32)
            cw2 = cload("cw2", (P, 128), d_cw2)
            cb2 = cload("cb2", (20, 1), d_cb2, f32)
            outsb = cp.tile([20, b_core], f32, tag="outsb", name="outsb")
            qoht = cp.tile([P, G * 8], f16, tag="qoh", name="qoh")
            nc.sync.dma_start(qoht[:], d_qoh[:])
            for g in range(G):
                QOH[g] = qoht[:, g * 8:(g + 1) * 8]

            stsbig = cp.tile([P, G * P], f16, tag="stsbig", name="stsbig")
            mm = nc.tensor.matmul

            for t_step in range(n_steps):
                for pb in range(NPAIR // 2):
                  for pi in (0, 1):
                    p = 2 * pb + pi
                    fat = caps[2 * p] == 256
                    aggt = pG.tile([P, 256], f32, tag="agg", name="agg")
                    agg = aggt[:]
                    # A = [S@W1a | S@W1c] slot-major, one tile per group
                    asb = wp.tile([P, 1024], f16, tag="asb", name="asb")
                    for gi in (0, 1):
                        aps = pA.tile([P, 512], f32, tag="aps", name="aps")
                        mm(aps[:], lhsT=S[p][:, gi * P:(gi + 1) * P], rhs=w1ac[:],
                           start=True, stop=True)
                        nc.vector.tensor_copy(asb[:, gi * 512:(gi + 1) * 512], aps[:])

                    if not fat:
                        # h1 for both groups in one 1-bank tile; per group:
                        # one rel MM (band trick: rows 0:20 F0 table, 32:52 F1)
                        # + 4 gather MMs. Each group's region closes before the
                        # next group's start (start clears bank has_written).
                        h1 = pH1.tile([P, 512], f32, tag="h1", name="h1")
                        for gi in (0, 1):
                            g = 2 * p + gi
                            base = gi * 256
                            mm(h1[:, base:base + 256], lhsT=rt2[:],
                               rhs=REL[g], start=True, stop=False)
                            for F in range(2):
                                o = h1[:, base + F * P:base + (F + 1) * P]
                                mm(o, lhsT=asb[:, gi * 512 + F * P:gi * 512 + (F + 1) * P],
                                   rhs=OHS[g], start=False, stop=False)
                                mm(o, lhsT=asb[:, gi * 512 + 256 + F * P:gi * 512 + 256 + (F + 1) * P],
                                   rhs=OHT[g], start=False, stop=True)
                        h1g = wp.tile([P, 512], f16, tag="h1g", name="h1g")
                        nc.scalar.activation(h1g[:], h1[:], AF.Gelu)
                        # msg layer 2, edge-major (pos, D), both groups
                        msg = pM.tile([P, 256], f32, tag="msg", name="msg")
                        for gi in (0, 1):
                            o = msg[:, gi * P:(gi + 1) * P]
                            for F in range(2):
                                mm(o, lhsT=h1g[:, gi * 256 + F * P:gi * 256 + (F + 1) * P],
                                   rhs=w2m[:, F * P:(F + 1) * P],
                                   start=(F == 0), stop=(F == 1))
                        msb = wp.tile([P, 256], f16, tag="msb", name="msb")
                        nc.scalar.copy(msb[:], msg[:])
                        for gi in (0, 1):
                            mm(agg[:, gi * P:(gi + 1) * P],
                               lhsT=msb[:, gi * P:(gi + 1) * P], rhs=OHE[2 * p + gi],
                               start=True, stop=True)
                    else:
                        for gi in (0, 1):
                            g = 2 * p + gi
                            h1 = pH1.tile([P, 512], f32, tag="h1", name="h1")
                            mm(h1[:], lhsT=rt2[:], rhs=REL[g][:],
                               start=True, stop=False)
                            for F in range(2):
                                o = h1[:, F * 256:(F + 1) * 256]
                                mm(o, lhsT=asb[:, gi * 512 + F * P:gi * 512 + (F + 1) * P],
                                   rhs=OHS[g], start=False, stop=False)
                                mm(o, lhsT=asb[:, gi * 512 + 256 + F * P:gi * 512 + 256 + (F + 1) * P],
                                   rhs=OHT[g], start=False, stop=True)
                            h1g = wp.tile([P, 512], f16, tag="h1g", name="h1g")
                            nc.scalar.activation(h1g[:], h1[:], AF.Gelu)
                            msg = pM.tile([P, 256], f32, tag="msg", name="msg")
                            for ec in range(2):
                                o = msg[:, ec * P:(ec + 1) * P]
                                for F in range(2):
                                    mm(o, lhsT=h1g[:, F * 256 + ec * P:F * 256 + (ec + 1) * P],
                                       rhs=w2m[:, F * P:(F + 1) * P],
                                       start=(F == 0), stop=(F == 1))
                            msb = wp.tile([P, 256], f16, tag="msb", name="msb")
                            nc.scalar.copy(msb[:], msg[:])
                            o = agg[:, gi * P:(gi + 1) * P]
                            for ec in range(2):
                                mm(o, lhsT=msb[:, ec * P:(ec + 1) * P],
                                   rhs=OHE[g][:, ec * P:(ec + 1) * P],
                                   start=(ec == 0), stop=(ec == 1))
                    # gsb = agg + msg_b2 * indegree (DVE)
                    gsbt = wp.tile([P, 256], f16, tag="gsb", name="gsb")
                    nc.vector.scalar_tensor_tensor(
                        out=gsbt[:], in0=INDB[2 * pb][:, pi * 256:(pi + 1) * 256],
                        scalar=b2m[:, 0:1],
                        in1=agg, op0=OP.mult, op1=OP.add)
                    gsb = gsbt[:]
                    # update MLP over the pair (256 slot cols)
                    h3 = pH3.tile([P, 512], f32, tag="h3", name="h3")
                    for mc in range(2):
                        o = h3[:, mc * 256:(mc + 1) * 256]
                        mm(o, lhsT=w1u[:, mc * P:(mc + 1) * P], rhs=S[p],
                           start=True, stop=False)
                        mm(o, lhsT=w1u[:, 256 + mc * P:256 + (mc + 1) * P], rhs=gsb,
                           start=False, stop=True)
                    h3g = wp.tile([P, 512], f16, tag="h3g", name="h3g")
                    for mc in range(2):
                        nc.scalar.activation(
                            h3g[:, mc * 256:(mc + 1) * 256],
                            h3[:, mc * 256:(mc + 1) * 256], AF.Gelu,
                            bias=b1u[:, mc:mc + 1])
                    sn = pS.tile([P, 256], f32, tag="sn", name="sn")
                    for kc in range(2):
                        mm(sn[:], lhsT=w2u[:, kc * P:(kc + 1) * P],
                           rhs=h3g[:, kc * 256:(kc + 1) * 256],
                           start=(kc == 0), stop=(kc == 1))
                    # S += sn + b2u, f16 in place
                    nc.vector.scalar_tensor_tensor(
                        out=S[p], in0=sn[:], scalar=b2u[:, 0:1], in1=S[p],
                        op0=OP.add, op1=OP.add)
                    if t_step == n_steps - 1:
                        for gi in (0, 1):
                            g = 2 * p + gi
                            nc.sync.dma_start_transpose(
                                stsbig[:, g * P:(g + 1) * P],
                                S[p][:, gi * P:(gi + 1) * P])

            if debug_dump:
                for p in range(NPAIR):
                    nc.sync.dma_start(d_sdump[:, p * 256:(p + 1) * 256], S[p])
            # classifier head (f16 matmuls)
            nbatch = (G + 15) // 16
            for bq in range(nbatch):
                jn = min(16, G - bq * 16)
                qps = pH1.tile([P, P], f32, tag="h1", name="qps")
                for j in range(jn):
                    g = bq * 16 + j
                    mm(qps[:, j * 8:(j + 1) * 8], lhsT=stsbig[:, g * P:(g + 1) * P],
                       rhs=QOH[g], start=True, stop=True)
                qcat = wp.tile([P, P], f16, tag="qcat", name="qcat")
                nc.vector.tensor_copy(qcat[:, 0:jn * 8], qps[:, 0:jn * 8])
                qv = qcat[:, 0:jn * 8].rearrange("p (g t f) -> p g t f", t=2, f=4)
                ncols = jn * 4
                hps = pM.tile([P, ncols], f32, tag="msg", name="hps",
                              padded_shape=[P, 256])
                mm(hps[:], lhsT=cw1[:, 0:P], rhs=qv[:, :, 0, :], start=True, stop=False)
                mm(hps[:], lhsT=cw1[:, P:256], rhs=qv[:, :, 1, :], start=False, stop=True)
                hg = wp.tile([P, ncols], f16, tag="hg", name="hg",
                             padded_shape=[P, 64])
                nc.scalar.activation(hg[:], hps[:], AF.Gelu, bias=cb1[:, 0:1])
                ops_ = pG.tile([P, ncols], f32, tag="agg", name="ops",
                               padded_shape=[P, 256])
                mm(ops_[:], lhsT=cw2[:], rhs=hg[:], start=True, stop=True)
                nc.scalar.activation(
                    outsb[:, bq * 64:bq * 64 + ncols], ops_[0:20, :], AF.Identity,
                    bias=cb2[:, 0:1])
            nc.sync.dma_start(d_out[:], outsb[:])

    nc.finalize()
    return nc


def _pack_samples(ne, b_core):
    """Global balanced assignment: returns per-core list of
    (group, j) -> original sample index arrays, shape (n_cores, G, 4)."""
    B = ne.shape[0]
    n_cores = B // b_core
    G = b_core // GRP
    caps = _caps_for(b_core)
    n_thin = sum(1 for c in caps if c == 128)
    nfat = G - n_thin

    order = np.argsort(-ne, kind="stable")
    core_sum = np.zeros(n_cores, dtype=np.int64)
    core_cnt = np.zeros(n_cores, dtype=np.int64)
    core_members = [[] for _ in range(n_cores)]
    for s in order:
        avail = np.where(core_cnt < b_core)[0]
        c = avail[np.argmin(core_sum[avail])]
        core_members[c].append(s)
        core_sum[c] += ne[s]
        core_cnt[c] += 1

    layout = np.zeros((n_cores, G, GRP), dtype=np.int64)
    ok = True
    for c in range(n_cores):
        mem = np.asarray(core_members[c])
        o = np.argsort(-ne[mem], kind="stable")
        mem = mem[o]
        fat = mem[:4 * nfat]
        rest = mem[4 * nfat:]
        # fat groups occupy the END of the group list (indices n_thin..G-1)
        for fg in range(nfat):
            layout[c, n_thin + fg] = fat[fg * 4:(fg + 1) * 4]
        nr = ne[rest]
        sums = np.zeros(n_thin, dtype=np.int64)
        cnt = np.zeros(n_thin, dtype=np.int64)
        for k in range(rest.shape[0]):
            avail = np.where(cnt < GRP)[0]
            gsel = avail[np.argmin(sums[avail])]
            layout[c, gsel, cnt[gsel]] = rest[k]
            sums[gsel] += nr[k]
            cnt[gsel] += 1
        if n_thin and sums.max() > 128:
            ok = False
    return layout, ok


def _host_prep(inputs, b_core):
    f = np.float32
    h = np.float16
    src_a = np.asarray(inputs["edge_src"]).astype(np.int64)
    tgt_a = np.asarray(inputs["edge_tgt"]).astype(np.int64)
    rel_a = np.asarray(inputs["edge_rel"]).astype(np.int64)
    ne = np.asarray(inputs["n_edges"]).astype(np.int64)
    qs_a = np.asarray(inputs["query_src"]).astype(np.int64)
    qt_a = np.asarray(inputs["query_tgt"]).astype(np.int64)
    B = src_a.shape[0]
    n_cores = B // b_core
    G = b_core // GRP
    NPAIR = G // 2
    caps = _caps_for(b_core)
    n_thin = sum(1 for c in caps if c == 128)
    nfat = G - n_thin
    n_reltile = (n_thin + 3) // 4

    layout, ok = _pack_samples(ne, b_core)
    if not ok:
        # fall back: no thin groups (callers rebuild nc keyed by caps anyway)
        raise RuntimeError("edge packing overflow; unsupported input")

    # shared params
    ee = np.asarray(inputs["entity_embed"], f)
    w1 = np.asarray(inputs["msg_W1"], f)
    reltab = np.asarray(inputs["rel_embed"], f) @ w1[128:256] + np.asarray(inputs["msg_b1"], f)
    # band layout: rows 0:20 hold the F0 half of the rel table, rows 32:52 the
    # F1 half; the rel one-hot rhs selects the right band per column range
    rt2 = np.zeros((P, 128), f)
    rt2[0:20] = reltab[:, 0:128]
    rt2[32:52] = reltab[:, 128:256]
    w2m_ = np.asarray(inputs["msg_W2"], f)
    w1u_ = np.asarray(inputs["upd_W1"], f)
    w2u_ = np.asarray(inputs["upd_W2"], f)
    cw1_ = np.asarray(inputs["cls_W1"], f)
    cw2p = np.zeros((P, 128), f)
    cw2p[:, 0:20] = np.asarray(inputs["cls_W2"], f)
    shared = {
        "s0": np.ascontiguousarray(ee.T.astype(h)),
        "rt2": rt2.astype(h),
        "b2m": np.asarray(inputs["msg_b2"], f).reshape(128, 1).copy(),
        "w1ac": np.concatenate([w1[0:128], w1[256:384]], axis=1).astype(h),
        "w2m": np.concatenate([w2m_[0:128], w2m_[128:256]], axis=1).astype(h),
        "w1u": np.concatenate(
            [w1u_[0:128, 0:128], w1u_[0:128, 128:256],
             w1u_[128:256, 0:128], w1u_[128:256, 128:256]], axis=1).astype(h),
        "w2u": np.concatenate([w2u_[0:128], w2u_[128:256]], axis=1).astype(h),
        "b1u": np.asarray(inputs["upd_b1"], f).reshape(2, 128).T.copy(),
        "b2u": np.asarray(inputs["upd_b2"], f).reshape(128, 1).copy(),
        "cw1": np.concatenate([cw1_[0:128], cw1_[128:256]], axis=1).astype(h),
        "cb1": np.asarray(inputs["cls_b1"], f).reshape(128, 1).copy(),
        "cw2": cw2p.astype(h),
        "cb2": np.asarray(inputs["cls_b2"], f).reshape(20, 1).copy(),
    }

    n_thinp = n_thin // 2
    n_fatp = nfat // 2
    in_maps = []
    perm = np.zeros(B, dtype=np.int64)  # out row (c*b_core + g*4 + j) -> sample
    for c in range(n_cores):
        bund_t = np.zeros((max(1, n_thinp), P, 1280), h)
        bund_f = np.zeros((max(1, n_fatp), P, 2560), h)
        ind = np.zeros((NPAIR, 256), np.float32)
        qoh = np.zeros((P, G * 8), h)

        thin_p = 0
        fat_p = 0
        for p in range(NPAIR):
            fatpair = caps[2 * p] == 256
            if fatpair:
                bund = bund_f[fat_p]
                fat_p += 1
            else:
                bund = bund_t[thin_p]
                thin_p += 1
            cap = caps[2 * p]
            cw = cap
            for gi in (0, 1):
                g = 2 * p + gi
                rtile = bund[:, gi * 2 * cw:(gi + 1) * 2 * cw]
                ohs = bund[:, 4 * cw + gi * cw:4 * cw + (gi + 1) * cw]
                oht = bund[:, 6 * cw + gi * cw:6 * cw + (gi + 1) * cw]
                ohe = bund[:, 8 * cw + gi * cw:8 * cw + (gi + 1) * cw]
                pos = 0
                for j in range(GRP):
                    s = layout[c, g, j]
                    perm[c * b_core + g * GRP + j] = s
                    nedge = ne[s]
                    for e in range(nedge):
                        ohs[j * 32 + src_a[s, e], pos] = 1
                        oht[j * 32 + tgt_a[s, e], pos] = 1
                        ec, pr = divmod(pos, 128)
                        ohe[pr, ec * 128 + j * 32 + tgt_a[s, e]] = 1
                        rtile[rel_a[s, e], pos] = 1
                        rtile[32 + rel_a[s, e], cap + pos] = 1
                        ind[p, gi * 128 + j * 32 + tgt_a[s, e]] += 1
                        pos += 1
                    assert pos <= cap
                    qoh[j * 32 + qs_a[s], g * 8 + j] = 1
                    qoh[j * 32 + qt_a[s], g * 8 + 4 + j] = 1

        indb = np.ascontiguousarray(
            np.broadcast_to(ind[:, None, :], (NPAIR, P, 256))
            .reshape(NPAIR // 8, 8, P, 256).transpose(0, 2, 1, 3)
            .reshape(NPAIR // 8, P, 8 * 256)).astype(h)
        m = dict(shared)
        if n_thinp:
            m["bund_t"] = bund_t
        if n_fatp:
            m["bund_f"] = bund_f
        m.update({"indb": indb, "qoh": qoh})
        in_maps.append(m)
    return in_maps, perm


_CACHE = {}


def _make_in_maps(inputs, b_core):
    return _host_prep(inputs, b_core)[0]


def kernel(**inputs):
    b = np.asarray(inputs["edge_src"]).shape[0]
    b_core = b // N_CORES
    _patch_ldw_opt()
    if b_core not in _CACHE:
        _CACHE[b_core] = _build_nc(b_core, N_STEPS)
    nc = _CACHE[b_core]

    in_maps, perm = _host_prep(inputs, b_core)

    from concourse.bass_utils import run_bass_kernel_spmd
    res = run_bass_kernel_spmd(nc, in_maps, core_ids=list(range(N_CORES)))
    packed = np.concatenate([r["out"].T for r in res.results], axis=0)
    out = np.zeros_like(packed)
    out[perm] = packed
    return np.ascontiguousarray(out, dtype=np.float32)


# revision 4
# speedup vs baseline: 1.0202x; 1.0152x over previous
"""Trainium2 Bass kernel: CLUTRR-style GNN message passing (nn_CLUTRRV4).

Data-parallel across 8 NeuronCores; per core 256 samples are packed
4-per-group (4 x 32 entity slots = 128 partitions), entity states stay
SBUF-resident (f16) for all 8 message-passing steps, and gather/scatter are
one-hot matmuls. Main optimizations vs the dense f32-master baseline
(1.56 ms -> ~0.64 ms):
  * One-hot gather/scatter/rel matrices precomputed on the HOST and DMA'd
    once (constant across steps) -- removes ~800us/core of DVE is_equal.
  * Edge sparsity packing: samples re-assigned globally (balanced core
    totals), then grouped per core so each thin group's VALID edges fit in
    128 columns (vs 256 dense): 62 thin groups + 1 fat pair (cap 256,
    absorbing the largest samples) per core; output un-permuted on host.
  * rel-table contribution via a single K=128 matmul per group using
    disjoint row bands (rows 0:20 = F0 table, 32:52 = F1) selected by the
    one-hot rhs column range.
  * msg_b2 * indegree folded into the agg->SBUF move as one DVE
    scalar_tensor_tensor; S updated in-place in f16.
  * Pair-merged PSUM tiles so each Scalar/Vector move or gelu covers a
    whole pair (halves per-instruction fixed overheads).
  * Inputs bundled into ~55 large DMAs split across both HWDGE queues
    (sync + scalar); S0 built on-chip by broadcast copy.
  * Classifier: per-group S^T via DMA-xbar transposes overlapped with
    step 7, f16 head matmuls, padded cw2 (M=128).
Engines end up balanced: PE ~595us (LDWEIGHTS-rate bound, the structural
floor: every matmul loads a fresh 128-col weight at 1.2 GHz; the compiler's
--enable-ldw-opt/FWL path is broken in this toolchain), Scalar ~565us,
Vector ~546us over a ~642us span.
"""
import sys
import numpy as np

if "/opt/trn_rl_repo" not in sys.path:
    sys.path.append("/opt/trn_rl_repo")

N_ENT, N_REL, D, E = 32, 20, 128, 64
N_STEPS = 8
N_CORES = 8
P = 128
GRP = 4  # samples per group


def _patch_ldw_opt():
    import os
    if os.environ.get("BASS_LDW_OPT") != "1":
        return
    from concourse import bass_utils as bu
    if getattr(bu, "_ldw_opt_patched", False):
        return
    orig = bu.run_command

    def run_command_ldw(cmd, *a, **kw):
        if isinstance(cmd, list):
            cmd = [c.replace("--enable-ldw-opt=false", "--enable-ldw-opt=true")
                   if isinstance(c, str) else c for c in cmd]
        return orig(cmd, *a, **kw)

    bu.run_command = run_command_ldw
    bu._ldw_opt_patched = True


def _caps_for(b_core):
    """Per-group edge capacities. 256 samples -> 62 thin (128) + 2 fat (256).
    Other sizes -> all-fat fallback (always packable: 4*63 <= 256)."""
    G = b_core // GRP
    if b_core == 256:
        return [128] * 62 + [256, 256]
    return [256] * G


def _build_nc(b_core, n_steps, debug_dump=False):
    from concourse import bacc, mybir
    from concourse.tile import TileContext
    from concourse.masks import make_identity

    f32 = mybir.dt.float32
    f16 = mybir.dt.float16
    AF = mybir.ActivationFunctionType
    OP = mybir.AluOpType

    caps = _caps_for(b_core)
    G = b_core // GRP
    NPAIR = G // 2
    n_thin = sum(1 for c in caps if c == 128)
    n_reltile = (n_thin + 3) // 4
    nfat = G - n_thin

    nc = bacc.Bacc()

    def din(name, shape, dtype=f16):
        return nc.declare_dram_parameter(name, list(shape), dtype, isOutput=False)

    # bundled per-pair one-hot block:
    # [rel-g0 | rel-g1 | ohs-g0 | ohs-g1 | oht-g0 | oht-g1 | ohe-g0 | ohe-g1]
    # thin pair: 2*256 + 6*128 = 1280 cols; fat pair: 2*512 + 6*256 = 2560
    n_thinp = n_thin // 2
    n_fatp = nfat // 2
    d_s0 = din("s0", (P, 32))
    d_bund_t = din("bund_t", (n_thinp, P, 1280)) if n_thinp else None
    d_bund_f = din("bund_f", (n_fatp, P, 2560)) if n_fatp else None
    d_indb = din("indb", (NPAIR // 8, P, 8 * 256))
    d_qoh = din("qoh", (P, G * 8))
    d_rt2 = din("rt2", (P, 128))
    d_b2m = din("b2m", (P, 1), f32)
    d_w1ac = din("w1ac", (P, 512))
    d_w2m = din("w2m", (P, 256))
    d_w1u = din("w1u", (P, 512))
    d_w2u = din("w2u", (P, 256))
    d_b1u = din("b1u", (P, 2), f32)
    d_b2u = din("b2u", (P, 1), f32)
    d_cw1 = din("cw1", (P, 256))
    d_cb1 = din("cb1", (P, 1), f32)
    d_cw2 = din("cw2", (P, 128))
    d_cb2 = din("cb2", (20, 1), f32)
    d_out = nc.declare_dram_parameter("out", [20, b_core], f32, isOutput=True)
    d_sdump = (nc.declare_dram_parameter("sdump", [P, (b_core // GRP) * P], f16,
                                         isOutput=True) if debug_dump else None)

    with TileContext(nc) as tc:
        with (
            tc.tile_pool(name="c", bufs=1) as cp,
            tc.tile_pool(name="w", bufs=4) as wp,
            tc.tile_pool(name="pA", bufs=2, space="PSUM") as pA,
            tc.tile_pool(name="pH1", bufs=2, space="PSUM") as pH1,
            tc.tile_pool(name="pM", bufs=1, space="PSUM") as pM,
            tc.tile_pool(name="pG", bufs=1, space="PSUM") as pG,
            tc.tile_pool(name="pH3", bufs=1, space="PSUM") as pH3,
            tc.tile_pool(name="pS", bufs=1, space="PSUM") as pS,
        ):
            def cload(name, shape, dram, dtype=f16):
                t = cp.tile(list(shape), dtype, tag=name, name=name)
                nc.sync.dma_start(t[:], dram[:])
                return t

            S = [None] * NPAIR
            Spair2 = [None] * (NPAIR // 2)
            OHS, OHT, OHE, QOH, REL = ([None] * G for _ in range(5))
            INDB = [None] * NPAIR
            IBT = [None] * (NPAIR // 8)
            fat_flags = [caps[2 * p] == 256 for p in range(NPAIR)]
            thin_idx = np.cumsum([0] + [0 if f else 1 for f in fat_flags]).tolist()
            fat_idx = np.cumsum([0] + [1 if f else 0 for f in fat_flags]).tolist()

            s0seed = cp.tile([P, 32], f16, tag="s0seed", name="s0seed")
            nc.sync.dma_start(s0seed[:], d_s0[:])

            def load_pair(p):
                if p % 2 == 0:
                    t = cp.tile([P, 512], f16, tag=f"S{p // 2}", name=f"S{p // 2}")
                    nc.vector.tensor_copy(
                        t[:].rearrange("p (r c) -> p r c", c=32),
                        s0seed[:, None, :].to_broadcast([P, 16, 32]))
                    Spair2[p // 2] = t[:]
                S[p] = Spair2[p // 2][:, (p % 2) * 256:(p % 2 + 1) * 256]
                if p % 8 == 0:
                    t = cp.tile([P, 8 * 256], f16, tag=f"ib{p // 8}",
                                name=f"ib{p // 8}")
                    nc.sync.dma_start(t[:], d_indb[p // 8])
                    IBT[p // 8] = t
                if p % 2 == 0:
                    INDB[p] = IBT[p // 8][:, (p % 8) * 256:(p % 8 + 2) * 256]
                fat = fat_flags[p]
                w = 2560 if fat else 1280
                bund = cp.tile([P, w], f16, tag=f"bk{p}", name=f"bk{p}")
                dma_eng = nc.sync if p % 2 == 0 else nc.scalar
                if fat:
                    dma_eng.dma_start(bund[:], d_bund_f[fat_idx[p]])
                else:
                    dma_eng.dma_start(bund[:], d_bund_t[thin_idx[p]])
                cw = w // 10  # 128 thin, 256 fat (per-group one-hot width)
                for gi in (0, 1):
                    g = 2 * p + gi
                    REL[g] = bund[:, gi * 2 * cw:(gi + 1) * 2 * cw]
                    OHS[g] = bund[:, 4 * cw + gi * cw:4 * cw + (gi + 1) * cw]
                    OHT[g] = bund[:, 6 * cw + gi * cw:6 * cw + (gi + 1) * cw]
                    OHE[g] = bund[:, 8 * cw + gi * cw:8 * cw + (gi + 1) * cw]

            # step-0 critical-path weights + first pairs' data first
            w1ac = cload("w1ac", (P, 512), d_w1ac)
            rt2 = cload("rt2", (P, 128), d_rt2)
            load_pair(0)
            load_pair(1)
            w2m = cload("w2m", (P, 256), d_w2m)
            w1u = cload("w1u", (P, 512), d_w1u)
            w2u = cload("w2u", (P, 256), d_w2u)
            b2m = cload("b2m", (P, 1), d_b2m, f32)
            b1u = cload("b1u", (P, 2), d_b1u, f32)
            b2u = cload("b2u", (P, 1), d_b2u, f32)
            for p in range(2, NPAIR):
                load_pair(p)
            cw1 = cload("cw1", (P, 256), d_cw1)
            cb1 = cload("cb1", (P, 1), d_cb1, f32)
            cw2 = cload("cw2", (P, 128), d_cw2)
            cb2 = cload("cb2", (20, 1), d_cb2, f32)
            outsb = cp.tile([20, b_core], f32, tag="outsb", name="outsb")
            qoht = cp.tile([P, G * 8], f16, tag="qoh", name="qoh")
            nc.sync.dma_start(qoht[:], d_qoh[:])
            for g in range(G):
                QOH[g] = qoht[:, g * 8:(g + 1) * 8]

            stsbig = cp.tile([P, G * P], f16, tag="stsbig", name="stsbig")
            mm = nc.tensor.matmul

            for t_step in range(n_steps):
                for pb in range(NPAIR // 2):
                  for pi in (0, 1):
                    p = 2 * pb + pi
                    fat = caps[2 * p] == 256
                    aggt = pG.tile([P, 256], f32, tag="agg", name="agg")
                    agg = aggt[:]
                    # A = [S@W1a | S@W1c] slot-major, one tile per group
                    asb = wp.tile([P, 1024], f16, tag="asb", name="asb")
                    for gi in (0, 1):
                        aps = pA.tile([P, 512], f32, tag="aps", name="aps")
                        mm(aps[:], lhsT=S[p][:, gi * P:(gi + 1) * P], rhs=w1ac[:],
                           start=True, stop=True)
                        nc.vector.tensor_copy(asb[:, gi * 512:(gi + 1) * 512], aps[:])

                    if not fat:
                        # h1 for both groups in one 1-bank tile; per group:
                        # one rel MM (band trick: rows 0:20 F0 table, 32:52 F1)
                        # + 4 gather MMs. Each group's region closes before the
                        # next group's start (start clears bank has_written).
                        h1 = pH1.tile([P, 512], f32, tag="h1", name="h1")
                        for gi in (0, 1):
                            g = 2 * p + gi
                            base = gi * 256
                            mm(h1[:, base:base + 256], lhsT=rt2[:],
                               rhs=REL[g], start=True, stop=False)
                            for F in range(2):
                                o = h1[:, base + F * P:base + (F + 1) * P]
                                mm(o, lhsT=asb[:, gi * 512 + F * P:gi * 512 + (F + 1) * P],
                                   rhs=OHS[g], start=False, stop=False)
                                mm(o, lhsT=asb[:, gi * 512 + 256 + F * P:gi * 512 + 256 + (F + 1) * P],
                                   rhs=OHT[g], start=False, stop=True)
                        h1g = wp.tile([P, 512], f16, tag="h1g", name="h1g")
                        nc.scalar.activation(h1g[:], h1[:], AF.Gelu)
                        # msg layer 2, edge-major (pos, D), both groups
                        msg = pM.tile([P, 256], f32, tag="msg", name="msg")
                        for gi in (0, 1):
                            o = msg[:, gi * P:(gi + 1) * P]
                            for F in range(2):
                                mm(o, lhsT=h1g[:, gi * 256 + F * P:gi * 256 + (F + 1) * P],
                                   rhs=w2m[:, F * P:(F + 1) * P],
                                   start=(F == 0), stop=(F == 1))
                        msb = wp.tile([P, 256], f16, tag="msb", name="msb")
                        nc.scalar.copy(msb[:], msg[:])
                        for gi in (0, 1):
                            mm(agg[:, gi * P:(gi + 1) * P],
                               lhsT=msb[:, gi * P:(gi + 1) * P], rhs=OHE[2 * p + gi],
                               start=True, stop=True)
                    else:
                        for gi in (0, 1):
                            g = 2 * p + gi
                            h1 = pH1.tile([P, 512], f32, tag="h1", name="h1")
                            mm(h1[:], lhsT=rt2[:], rhs=REL[g][:],
                               start=True, stop=False)
                            for F in range(2):
                                o = h1[:, F * 256:(F + 1) * 256]
                                mm(o, lhsT=asb[:, gi * 512 + F * P:gi * 512 + (F + 1) * P],
                                   rhs=OHS[g], start=False, stop=False)
                                mm(o, lhsT=asb[:, gi * 512 + 256 + F * P:gi * 512 + 256 + (F + 1) * P],
                                   rhs=OHT[g], start=False, stop=True)
                            h1g = wp.tile([P, 512], f16, tag="h1g", name="h1g")
                            nc.scalar.activation(h1g[:], h1[:], AF.Gelu)
                            msg = pM.tile([P, 256], f32, tag="msg", name="msg")
                            for ec in range(2):
                                o = msg[:, ec * P:(ec + 1) * P]
                                for F in range(2):
                                    mm(o, lhsT=h1g[:, F * 256 + ec * P:F * 256 + (ec + 1) * P],
                                       rhs=w2m[:, F * P:(F + 1) * P],
                                       start=(F == 0), stop=(F == 1))
                            msb = wp.tile([P, 256], f16, tag="msb", name="msb")
                            nc.scalar.copy(msb[:], msg[:])
                            o = agg[:, gi * P:(gi + 1) * P]
                            for ec in range(2):
                                mm(o, lhsT=msb[:, ec * P:(ec + 1) * P],
                                   rhs=OHE[g][:, ec * P:(ec + 1) * P],
                                   start=(ec == 0), stop=(ec == 1))
                    # gsb = agg + msg_b2 * indegree (DVE)
                    gsbt = wp.tile([P, 256], f16, tag="gsb", name="gsb")
                    nc.vector.scalar_tensor_tensor(
                        out=gsbt[:], in0=INDB[2 * pb][:, pi * 256:(pi + 1) * 256],
                        scalar=b2m[:, 0:1],
                        in1=agg, op0=OP.mult, op1=OP.add)
                    gsb = gsbt[:]
                    # update MLP over the pair (256 slot cols)
                    h3 = pH3.tile([P, 512], f32, tag="h3", name="h3")
                    for mc in range(2):
                        o = h3[:, mc * 256:(mc + 1) * 256]
                        mm(o, lhsT=w1u[:, mc * P:(mc + 1) * P], rhs=S[p],
                           start=True, stop=False)
                        mm(o, lhsT=w1u[:, 256 + mc * P:256 + (mc + 1) * P], rhs=gsb,
                           start=False, stop=True)
                    h3g = wp.tile([P, 512], f16, tag="h3g", name="h3g")
                    for mc in range(2):
                        nc.scalar.activation(
                            h3g[:, mc * 256:(mc + 1) * 256],
                            h3[:, mc * 256:(mc + 1) * 256], AF.Gelu,
                            bias=b1u[:, mc:mc + 1])
                    sn = pS.tile([P, 256], f32, tag="sn", name="sn")
                    for kc in range(2):
                        mm(sn[:], lhsT=w2u[:, kc * P:(kc + 1) * P],
                           rhs=h3g[:, kc * 256:(kc + 1) * 256],
                           start=(kc == 0), stop=(kc == 1))
                    # S += sn + b2u, f16 in place
                    nc.vector.scalar_tensor_tensor(
                        out=S[p], in0=sn[:], scalar=b2u[:, 0:1], in1=S[p],
                        op0=OP.add, op1=OP.add)
                    if t_step == n_steps - 1:
                        for gi in (0, 1):
                            g = 2 * p + gi
                            nc.sync.dma_start_transpose(
                                stsbig[:, g * P:(g + 1) * P],
                                S[p][:, gi * P:(gi + 1) * P])

            if debug_dump:
                for p in range(NPAIR):
                    nc.sync.dma_start(d_sdump[:, p * 256:(p + 1) * 256], S[p])
            # classifier head (f16 matmuls)
            nbatch = (G + 15) // 16
            for bq in range(nbatch):
                jn = min(16, G - bq * 16)
                qps = pH1.tile([P, P], f32, tag="h1", name="qps")
                for j in range(jn):
                    g = bq * 16 + j
                    mm(qps[:, j * 8:(j + 1) * 8], lhsT=stsbig[:, g * P:(g + 1) * P],
                       rhs=QOH[g], start=True, stop=True)
                qcat = wp.tile([P, P], f16, tag="qcat", name="qcat")
                nc.vector.tensor_copy(qcat[:, 0:jn * 8], qps[:, 0:jn * 8])
                qv = qcat[:, 0:jn * 8].rearrange("p (g t f) -> p g t f", t=2, f=4)
                ncols = jn * 4
                hps = pM.tile([P, ncols], f32, tag="msg", name="hps",
                              padded_shape=[P, 256])
                mm(hps[:], lhsT=cw1[:, 0:P], rhs=qv[:, :, 0, :], start=True, stop=False)
                mm(hps[:], lhsT=cw1[:, P:256], rhs=qv[:, :, 1, :], start=False, stop=True)
                hg = wp.tile([P, ncols], f16, tag="hg", name="hg",
                             padded_shape=[P, 64])
                nc.scalar.activation(hg[:], hps[:], AF.Gelu, bias=cb1[:, 0:1])
                ops_ = pG.tile([P, ncols], f32, tag="agg", name="ops",
                               padded_shape=[P, 256])
                mm(ops_[:], lhsT=cw2[:], rhs=hg[:], start=True, stop=True)
                nc.scalar.activation(
                    outsb[:, bq * 64:bq * 64 + ncols], ops_[0:20, :], AF.Identity,
                    bias=cb2[:, 0:1])
            nc.sync.dma_start(d_out[:], outsb[:])

    nc.finalize()
    return nc


def _pack_samples(ne, b_core):
    """Global balanced assignment: returns per-core list of
    (group, j) -> original sample index arrays, shape (n_cores, G, 4)."""
    B = ne.shape[0]
    n_cores = B // b_core
    G = b_core // GRP
    caps = _caps_for(b_core)
    n_thin = sum(1 for c in caps if c == 128)
    nfat = G - n_thin

    order = np.argsort(-ne, kind="stable")
    core_sum = np.zeros(n_cores, dtype=np.int64)
    core_cnt = np.zeros(n_cores, dtype=np.int64)
    core_members = [[] for _ in range(n_cores)]
    for s in order:
        avail = np.where(core_cnt < b_core)[0]
        c = avail[np.argmin(core_sum[avail])]
        core_members[c].append(s)
        core_sum[c] += ne[s]
        core_cnt[c] += 1

    layout = np.zeros((n_cores, G, GRP), dtype=np.int64)
    ok = True
    for c in range(n_cores):
        mem = np.asarray(core_members[c])
        o = np.argsort(-ne[mem], kind="stable")
        mem = mem[o]
        fat = mem[:4 * nfat]
        rest = mem[4 * nfat:]
        # fat groups occupy the END of the group list (indices n_thin..G-1)
        for fg in range(nfat):
            layout[c, n_thin + fg] = fat[fg * 4:(fg + 1) * 4]
        nr = ne[rest]
        sums = np.zeros(n_thin, dtype=np.int64)
        cnt = np.zeros(n_thin, dtype=np.int64)
        for k in range(rest.shape[0]):
            avail = np.where(cnt < GRP)[0]
            gsel = avail[np.argmin(sums[avail])]
            layout[c, gsel, cnt[gsel]] = rest[k]
            sums[gsel] += nr[k]
            cnt[gsel] += 1
        if n_thin and sums.max() > 128:
            ok = False
    return layout, ok


def _host_prep(inputs, b_core):
    f = np.float32
    h = np.float16
    src_a = np.asarray(inputs["edge_src"]).astype(np.int64)
    tgt_a = np.asarray(inputs["edge_tgt"]).astype(np.int64)
    rel_a = np.asarray(inputs["edge_rel"]).astype(np.int64)
    ne = np.asarray(inputs["n_edges"]).astype(np.int64)
    qs_a = np.asarray(inputs["query_src"]).astype(np.int64)
    qt_a = np.asarray(inputs["query_tgt"]).astype(np.int64)
    B = src_a.shape[0]
    n_cores = B // b_core
    G = b_core // GRP
    NPAIR = G // 2
    caps = _caps_for(b_core)
    n_thin = sum(1 for c in caps if c == 128)
    nfat = G - n_thin
    n_reltile = (n_thin + 3) // 4

    layout, ok = _pack_samples(ne, b_core)
    if not ok:
        # fall back: no thin groups (callers rebuild nc keyed by caps anyway)
        raise RuntimeError("edge packing overflow; unsupported input")

    # shared params
    ee = np.asarray(inputs["entity_embed"], f)
    w1 = np.asarray(inputs["msg_W1"], f)
    reltab = np.asarray(inputs["rel_embed"], f) @ w1[128:256] + np.asarray(inputs["msg_b1"], f)
    # band layout: rows 0:20 hold the F0 half of the rel table, rows 32:52 the
    # F1 half; the rel one-hot rhs selects the right band per column range
    rt2 = np.zeros((P, 128), f)
    rt2[0:20] = reltab[:, 0:128]
    rt2[32:52] = reltab[:, 128:256]
    w2m_ = np.asarray(inputs["msg_W2"], f)
    w1u_ = np.asarray(inputs["upd_W1"], f)
    w2u_ = np.asarray(inputs["upd_W2"], f)
    cw1_ = np.asarray(inputs["cls_W1"], f)
    cw2p = np.zeros((P, 128), f)
    cw2p[:, 0:20] = np.asarray(inputs["cls_W2"], f)
    shared = {
        "s0": np.ascontiguousarray(ee.T.astype(h)),
        "rt2": rt2.astype(h),
        "b2m": np.asarray(inputs["msg_b2"], f).reshape(128, 1).copy(),
        "w1ac": np.concatenate([w1[0:128], w1[256:384]], axis=1).astype(h),
        "w2m": np.concatenate([w2m_[0:128], w2m_[128:256]], axis=1).astype(h),
        "w1u": np.concatenate(
            [w1u_[0:128, 0:128], w1u_[0:128, 128:256],
             w1u_[128:256, 0:128], w1u_[128:256, 128:256]], axis=1).astype(h),
        "w2u": np.concatenate([w2u_[0:128], w2u_[128:256]], axis=1).astype(h),
        "b1u": np.asarray(inputs["upd_b1"], f).reshape(2, 128).T.copy(),
        "b2u": np.asarray(inputs["upd_b2"], f).reshape(128, 1).copy(),
        "cw1": np.concatenate([cw1_[0:128], cw1_[128:256]], axis=1).astype(h),
        "cb1": np.asarray(inputs["cls_b1"], f).reshape(128, 1).copy(),
        "cw2": cw2p.astype(h),
        "cb2": np.asarray(inputs["cls_b2"], f).reshape(20, 1).copy(),
    }

    n_thinp = n_thin // 2
    n_fatp = nfat // 2
    in_maps = []
    perm = np.zeros(B, dtype=np.int64)  # out row (c*b_core + g*4 + j) -> sample
    for c in range(n_cores):
        bund_t = np.zeros((max(1, n_thinp), P, 1280), h)
        bund_f = np.zeros((max(1, n_fatp), P, 2560), h)
        ind = np.zeros((NPAIR, 256), np.float32)
        qoh = np.zeros((P, G * 8), h)

        thin_p = 0
        fat_p = 0
        for p in range(NPAIR):
            fatpair = caps[2 * p] == 256
            if fatpair:
                bund = bund_f[fat_p]
                fat_p += 1
            else:
                bund = bund_t[thin_p]
                thin_p += 1
            cap = caps[2 * p]
            cw = cap
            for gi in (0, 1):
                g = 2 * p + gi
                rtile = bund[:, gi * 2 * cw:(gi + 1) * 2 * cw]
                ohs = bund[:, 4 * cw + gi * cw:4 * cw + (gi + 1) * cw]
                oht = bund[:, 6 * cw + gi * cw:6 * cw + (gi + 1) * cw]
                ohe = bund[:, 8 * cw + gi * cw:8 * cw + (gi + 1) * cw]
                pos = 0
                for j in range(GRP):
                    s = layout[c, g, j]
                    perm[c * b_core + g * GRP + j] = s
                    nedge = ne[s]
                    for e in range(nedge):
                        ohs[j * 32 + src_a[s, e], pos] = 1
                        oht[j * 32 + tgt_a[s, e], pos] = 1
                        ec, pr = divmod(pos, 128)
                        ohe[pr, ec * 128 + j * 32 + tgt_a[s, e]] = 1
                        rtile[rel_a[s, e], pos] = 1
                        rtile[32 + rel_a[s, e], cap + pos] = 1
                        ind[p, gi * 128 + j * 32 + tgt_a[s, e]] += 1
                        pos += 1
                    assert pos <= cap
                    qoh[j * 32 + qs_a[s], g * 8 + j] = 1
                    qoh[j * 32 + qt_a[s], g * 8 + 4 + j] = 1

        indb = np.ascontiguousarray(
            np.broadcast_to(ind[:, None, :], (NPAIR, P, 256))
            .reshape(NPAIR // 8, 8, P, 256).transpose(0, 2, 1, 3)
            .reshape(NPAIR // 8, P, 8 * 256)).astype(h)
        m = dict(shared)
        if n_thinp:
            m["bund_t"] = bund_t
        if n_fatp:
            m["bund_f"] = bund_f
        m.update({"indb": indb, "qoh": qoh})
        in_maps.append(m)
    return in_maps, perm


_CACHE = {}


def _make_in_maps(inputs, b_core):
    return _host_prep(inputs, b_core)[0]


def kernel(**inputs):
    b = np.asarray(inputs["edge_src"]).shape[0]
    b_core = b // N_CORES
    _patch_ldw_opt()
    if b_core not in _CACHE:
        _CACHE[b_core] = _build_nc(b_core, N_STEPS)
    nc = _CACHE[b_core]

    in_maps, perm = _host_prep(inputs, b_core)

    from concourse.bass_utils import run_bass_kernel_spmd
    res = run_bass_kernel_spmd(nc, in_maps, core_ids=list(range(N_CORES)))
    packed = np.concatenate([r["out"].T for r in res.results], axis=0)
    out = np.zeros_like(packed)
    out[perm] = packed
    return np.ascontiguousarray(out, dtype=np.float32)


# revision 5
# speedup vs baseline: 1.0252x; 1.0049x over previous
"""Trainium2 Bass kernel: CLUTRR-style GNN message passing (nn_CLUTRRV4).

Data-parallel across 8 NeuronCores; per core 256 samples are packed
4-per-group (4 x 32 entity slots = 128 partitions), entity states stay
SBUF-resident (f16) for all 8 message-passing steps, and gather/scatter are
one-hot matmuls. Main optimizations vs the dense f32-master baseline
(1.56 ms -> ~0.63 ms):
  * One-hot gather/scatter/rel matrices precomputed on the HOST and DMA'd
    once (constant across steps) -- removes ~800us/core of DVE is_equal.
  * Edge sparsity packing: samples re-assigned globally (balanced core
    totals), then grouped per core so each thin group's VALID edges fit in
    128 columns (vs 256 dense): 62 thin groups + 1 fat pair (cap 256,
    absorbing the largest samples) per core; output un-permuted on host.
  * rel-table contribution via a single K=128 matmul per group using
    disjoint row bands (rows 0:20 = F0 table, 32:52 = F1) selected by the
    one-hot rhs column range.
  * msg_b2 * indegree folded into the agg->SBUF move as one DVE
    scalar_tensor_tensor; S updated in-place in f16.
  * Pair-merged PSUM tiles so each Scalar/Vector move or gelu covers a
    whole pair (halves per-instruction fixed overheads).
  * Inputs bundled into ~55 large DMAs split across both HWDGE queues
    (sync + scalar); S0 built on-chip by broadcast copy.
  * Classifier: per-group S^T via DMA-xbar transposes (even groups,
    overlapped with step 7) + PE transpose-mode (odd groups, in the
    otherwise-idle tail); f16 head matmuls, padded cw2 (M=128).
Engines end up balanced: PE ~595us (LDWEIGHTS-rate bound, the structural
floor: every matmul loads a fresh 128-col weight at 1.2 GHz; the compiler's
--enable-ldw-opt/FWL path is broken in this toolchain), Scalar ~565us,
Vector ~546us over a ~633us span.
"""
import sys
import numpy as np

if "/opt/trn_rl_repo" not in sys.path:
    sys.path.append("/opt/trn_rl_repo")

N_ENT, N_REL, D, E = 32, 20, 128, 64
N_STEPS = 8
N_CORES = 8
P = 128
GRP = 4  # samples per group


def _patch_ldw_opt():
    import os
    if os.environ.get("BASS_LDW_OPT") != "1":
        return
    from concourse import bass_utils as bu
    if getattr(bu, "_ldw_opt_patched", False):
        return
    orig = bu.run_command

    def run_command_ldw(cmd, *a, **kw):
        if isinstance(cmd, list):
            cmd = [c.replace("--enable-ldw-opt=false", "--enable-ldw-opt=true")
                   if isinstance(c, str) else c for c in cmd]
        return orig(cmd, *a, **kw)

    bu.run_command = run_command_ldw
    bu._ldw_opt_patched = True


def _caps_for(b_core):
    """Per-group edge capacities. 256 samples -> 62 thin (128) + 2 fat (256).
    Other sizes -> all-fat fallback (always packable: 4*63 <= 256)."""
    G = b_core // GRP
    if b_core == 256:
        return [128] * 62 + [256, 256]
    return [256] * G


def _build_nc(b_core, n_steps, debug_dump=False):
    from concourse import bacc, mybir
    from concourse.tile import TileContext
    from concourse.masks import make_identity

    f32 = mybir.dt.float32
    f16 = mybir.dt.float16
    AF = mybir.ActivationFunctionType
    OP = mybir.AluOpType

    caps = _caps_for(b_core)
    G = b_core // GRP
    NPAIR = G // 2
    n_thin = sum(1 for c in caps if c == 128)
    n_reltile = (n_thin + 3) // 4
    nfat = G - n_thin

    nc = bacc.Bacc()

    def din(name, shape, dtype=f16):
        return nc.declare_dram_parameter(name, list(shape), dtype, isOutput=False)

    # bundled per-pair one-hot block:
    # [rel-g0 | rel-g1 | ohs-g0 | ohs-g1 | oht-g0 | oht-g1 | ohe-g0 | ohe-g1]
    # thin pair: 2*256 + 6*128 = 1280 cols; fat pair: 2*512 + 6*256 = 2560
    n_thinp = n_thin // 2
    n_fatp = nfat // 2
    d_s0 = din("s0", (P, 32))
    d_bund_t = din("bund_t", (n_thinp, P, 1280)) if n_thinp else None
    d_bund_f = din("bund_f", (n_fatp, P, 2560)) if n_fatp else None
    d_indb = din("indb", (NPAIR // 8, P, 8 * 256))
    d_qoh = din("qoh", (P, G * 8))
    d_rt2 = din("rt2", (P, 128))
    d_b2m = din("b2m", (P, 1), f32)
    d_w1ac = din("w1ac", (P, 512))
    d_w2m = din("w2m", (P, 256))
    d_w1u = din("w1u", (P, 512))
    d_w2u = din("w2u", (P, 256))
    d_b1u = din("b1u", (P, 2), f32)
    d_b2u = din("b2u", (P, 1), f32)
    d_cw1 = din("cw1", (P, 256))
    d_cb1 = din("cb1", (P, 1), f32)
    d_cw2 = din("cw2", (P, 128))
    d_cb2 = din("cb2", (20, 1), f32)
    d_out = nc.declare_dram_parameter("out", [20, b_core], f32, isOutput=True)
    d_sdump = (nc.declare_dram_parameter("sdump", [P, (b_core // GRP) * P], f16,
                                         isOutput=True) if debug_dump else None)

    with TileContext(nc) as tc:
        with (
            tc.tile_pool(name="c", bufs=1) as cp,
            tc.tile_pool(name="w", bufs=4) as wp,
            tc.tile_pool(name="pA", bufs=2, space="PSUM") as pA,
            tc.tile_pool(name="pH1", bufs=2, space="PSUM") as pH1,
            tc.tile_pool(name="pM", bufs=1, space="PSUM") as pM,
            tc.tile_pool(name="pG", bufs=1, space="PSUM") as pG,
            tc.tile_pool(name="pH3", bufs=1, space="PSUM") as pH3,
            tc.tile_pool(name="pS", bufs=1, space="PSUM") as pS,
        ):
            def cload(name, shape, dram, dtype=f16):
                t = cp.tile(list(shape), dtype, tag=name, name=name)
                nc.sync.dma_start(t[:], dram[:])
                return t

            S = [None] * NPAIR
            Spair2 = [None] * (NPAIR // 2)
            OHS, OHT, OHE, QOH, REL = ([None] * G for _ in range(5))
            INDB = [None] * NPAIR
            IBT = [None] * (NPAIR // 8)
            fat_flags = [caps[2 * p] == 256 for p in range(NPAIR)]
            thin_idx = np.cumsum([0] + [0 if f else 1 for f in fat_flags]).tolist()
            fat_idx = np.cumsum([0] + [1 if f else 0 for f in fat_flags]).tolist()

            s0seed = cp.tile([P, 32], f16, tag="s0seed", name="s0seed")
            nc.sync.dma_start(s0seed[:], d_s0[:])

            def load_pair(p):
                if p % 2 == 0:
                    t = cp.tile([P, 512], f16, tag=f"S{p // 2}", name=f"S{p // 2}")
                    nc.vector.tensor_copy(
                        t[:].rearrange("p (r c) -> p r c", c=32),
                        s0seed[:, None, :].to_broadcast([P, 16, 32]))
                    Spair2[p // 2] = t[:]
                S[p] = Spair2[p // 2][:, (p % 2) * 256:(p % 2 + 1) * 256]
                if p % 8 == 0:
                    t = cp.tile([P, 8 * 256], f16, tag=f"ib{p // 8}",
                                name=f"ib{p // 8}")
                    nc.sync.dma_start(t[:], d_indb[p // 8])
                    IBT[p // 8] = t
                if p % 2 == 0:
                    INDB[p] = IBT[p // 8][:, (p % 8) * 256:(p % 8 + 2) * 256]
                fat = fat_flags[p]
                w = 2560 if fat else 1280
                bund = cp.tile([P, w], f16, tag=f"bk{p}", name=f"bk{p}")
                dma_eng = nc.sync if p % 2 == 0 else nc.scalar
                if fat:
                    dma_eng.dma_start(bund[:], d_bund_f[fat_idx[p]])
                else:
                    dma_eng.dma_start(bund[:], d_bund_t[thin_idx[p]])
                cw = w // 10  # 128 thin, 256 fat (per-group one-hot width)
                for gi in (0, 1):
                    g = 2 * p + gi
                    REL[g] = bund[:, gi * 2 * cw:(gi + 1) * 2 * cw]
                    OHS[g] = bund[:, 4 * cw + gi * cw:4 * cw + (gi + 1) * cw]
                    OHT[g] = bund[:, 6 * cw + gi * cw:6 * cw + (gi + 1) * cw]
                    OHE[g] = bund[:, 8 * cw + gi * cw:8 * cw + (gi + 1) * cw]

            # step-0 critical-path weights + first pairs' data first
            w1ac = cload("w1ac", (P, 512), d_w1ac)
            rt2 = cload("rt2", (P, 128), d_rt2)
            load_pair(0)
            load_pair(1)
            w2m = cload("w2m", (P, 256), d_w2m)
            w1u = cload("w1u", (P, 512), d_w1u)
            w2u = cload("w2u", (P, 256), d_w2u)
            b2m = cload("b2m", (P, 1), d_b2m, f32)
            b1u = cload("b1u", (P, 2), d_b1u, f32)
            b2u = cload("b2u", (P, 1), d_b2u, f32)
            for p in range(2, NPAIR):
                load_pair(p)
            cw1 = cload("cw1", (P, 256), d_cw1)
            cb1 = cload("cb1", (P, 1), d_cb1, f32)
            cw2 = cload("cw2", (P, 128), d_cw2)
            cb2 = cload("cb2", (20, 1), d_cb2, f32)
            outsb = cp.tile([20, b_core], f32, tag="outsb", name="outsb")
            qoht = cp.tile([P, G * 8], f16, tag="qoh", name="qoh")
            nc.sync.dma_start(qoht[:], d_qoh[:])
            for g in range(G):
                QOH[g] = qoht[:, g * 8:(g + 1) * 8]

            stsbig = cp.tile([P, G * P], f16, tag="stsbig", name="stsbig")
            ident = cp.tile([P, P], f16, tag="ident", name="ident")
            make_identity(nc, ident[:])
            mm = nc.tensor.matmul

            for t_step in range(n_steps):
                for pb in range(NPAIR // 2):
                  for pi in (0, 1):
                    p = 2 * pb + pi
                    fat = caps[2 * p] == 256
                    aggt = pG.tile([P, 256], f32, tag="agg", name="agg")
                    agg = aggt[:]
                    # A = [S@W1a | S@W1c] slot-major, one tile per group
                    asb = wp.tile([P, 1024], f16, tag="asb", name="asb")
                    for gi in (0, 1):
                        aps = pA.tile([P, 512], f32, tag="aps", name="aps")
                        mm(aps[:], lhsT=S[p][:, gi * P:(gi + 1) * P], rhs=w1ac[:],
                           start=True, stop=True)
                        nc.vector.tensor_copy(asb[:, gi * 512:(gi + 1) * 512], aps[:])

                    if not fat:
                        # h1 for both groups in one 1-bank tile; per group:
                        # one rel MM (band trick: rows 0:20 F0 table, 32:52 F1)
                        # + 4 gather MMs. Each group's region closes before the
                        # next group's start (start clears bank has_written).
                        h1 = pH1.tile([P, 512], f32, tag="h1", name="h1")
                        for gi in (0, 1):
                            g = 2 * p + gi
                            base = gi * 256
                            mm(h1[:, base:base + 256], lhsT=rt2[:],
                               rhs=REL[g], start=True, stop=False)
                            for F in range(2):
                                o = h1[:, base + F * P:base + (F + 1) * P]
                                mm(o, lhsT=asb[:, gi * 512 + F * P:gi * 512 + (F + 1) * P],
                                   rhs=OHS[g], start=False, stop=False)
                                mm(o, lhsT=asb[:, gi * 512 + 256 + F * P:gi * 512 + 256 + (F + 1) * P],
                                   rhs=OHT[g], start=False, stop=True)
                        h1g = wp.tile([P, 512], f16, tag="h1g", name="h1g")
                        nc.scalar.activation(h1g[:], h1[:], AF.Gelu)
                        # msg layer 2, edge-major (pos, D), both groups
                        msg = pM.tile([P, 256], f32, tag="msg", name="msg")
                        for gi in (0, 1):
                            o = msg[:, gi * P:(gi + 1) * P]
                            for F in range(2):
                                mm(o, lhsT=h1g[:, gi * 256 + F * P:gi * 256 + (F + 1) * P],
                                   rhs=w2m[:, F * P:(F + 1) * P],
                                   start=(F == 0), stop=(F == 1))
                        msb = wp.tile([P, 256], f16, tag="msb", name="msb")
                        nc.scalar.copy(msb[:], msg[:])
                        for gi in (0, 1):
                            mm(agg[:, gi * P:(gi + 1) * P],
                               lhsT=msb[:, gi * P:(gi + 1) * P], rhs=OHE[2 * p + gi],
                               start=True, stop=True)
                    else:
                        for gi in (0, 1):
                            g = 2 * p + gi
                            h1 = pH1.tile([P, 512], f32, tag="h1", name="h1")
                            mm(h1[:], lhsT=rt2[:], rhs=REL[g][:],
                               start=True, stop=False)
                            for F in range(2):
                                o = h1[:, F * 256:(F + 1) * 256]
                                mm(o, lhsT=asb[:, gi * 512 + F * P:gi * 512 + (F + 1) * P],
                                   rhs=OHS[g], start=False, stop=False)
                                mm(o, lhsT=asb[:, gi * 512 + 256 + F * P:gi * 512 + 256 + (F + 1) * P],
                                   rhs=OHT[g], start=False, stop=True)
                            h1g = wp.tile([P, 512], f16, tag="h1g", name="h1g")
                            nc.scalar.activation(h1g[:], h1[:], AF.Gelu)
                            msg = pM.tile([P, 256], f32, tag="msg", name="msg")
                            for ec in range(2):
                                o = msg[:, ec * P:(ec + 1) * P]
                                for F in range(2):
                                    mm(o, lhsT=h1g[:, F * 256 + ec * P:F * 256 + (ec + 1) * P],
                                       rhs=w2m[:, F * P:(F + 1) * P],
                                       start=(F == 0), stop=(F == 1))
                            msb = wp.tile([P, 256], f16, tag="msb", name="msb")
                            nc.scalar.copy(msb[:], msg[:])
                            o = agg[:, gi * P:(gi + 1) * P]
                            for ec in range(2):
                                mm(o, lhsT=msb[:, ec * P:(ec + 1) * P],
                                   rhs=OHE[g][:, ec * P:(ec + 1) * P],
                                   start=(ec == 0), stop=(ec == 1))
                    # gsb = agg + msg_b2 * indegree (DVE)
                    gsbt = wp.tile([P, 256], f16, tag="gsb", name="gsb")
                    nc.vector.scalar_tensor_tensor(
                        out=gsbt[:], in0=INDB[2 * pb][:, pi * 256:(pi + 1) * 256],
                        scalar=b2m[:, 0:1],
                        in1=agg, op0=OP.mult, op1=OP.add)
                    gsb = gsbt[:]
                    # update MLP over the pair (256 slot cols)
                    h3 = pH3.tile([P, 512], f32, tag="h3", name="h3")
                    for mc in range(2):
                        o = h3[:, mc * 256:(mc + 1) * 256]
                        mm(o, lhsT=w1u[:, mc * P:(mc + 1) * P], rhs=S[p],
                           start=True, stop=False)
                        mm(o, lhsT=w1u[:, 256 + mc * P:256 + (mc + 1) * P], rhs=gsb,
                           start=False, stop=True)
                    h3g = wp.tile([P, 512], f16, tag="h3g", name="h3g")
                    for mc in range(2):
                        nc.scalar.activation(
                            h3g[:, mc * 256:(mc + 1) * 256],
                            h3[:, mc * 256:(mc + 1) * 256], AF.Gelu,
                            bias=b1u[:, mc:mc + 1])
                    sn = pS.tile([P, 256], f32, tag="sn", name="sn")
                    for kc in range(2):
                        mm(sn[:], lhsT=w2u[:, kc * P:(kc + 1) * P],
                           rhs=h3g[:, kc * 256:(kc + 1) * 256],
                           start=(kc == 0), stop=(kc == 1))
                    # S += sn + b2u, f16 in place
                    nc.vector.scalar_tensor_tensor(
                        out=S[p], in0=sn[:], scalar=b2u[:, 0:1], in1=S[p],
                        op0=OP.add, op1=OP.add)
                    if t_step == n_steps - 1:
                        # even groups: xbar transpose on the sync DMA queue,
                        # overlapped with the rest of step 7; odd groups are
                        # transposed on the PE in the (otherwise idle) tail
                        for gi in (0, 1):
                            g = 2 * p + gi
                            if g % 2 == 0:
                                nc.sync.dma_start_transpose(
                                    stsbig[:, g * P:(g + 1) * P],
                                    S[p][:, gi * P:(gi + 1) * P])

            if debug_dump:
                for p in range(NPAIR):
                    nc.sync.dma_start(d_sdump[:, p * 256:(p + 1) * 256], S[p])
            # classifier head (f16 matmuls)
            nbatch = (G + 15) // 16
            for bq in range(nbatch):
                jn = min(16, G - bq * 16)
                qps = pH1.tile([P, P], f32, tag="h1", name="qps")
                for j in range(jn):
                    g = bq * 16 + j
                    if g % 2 == 1:
                        p2, gi = divmod(g, 2)
                        stp = pA.tile([P, P], f16, tag="aps", name="stp")
                        nc.tensor.transpose(stp[:], S[p2][:, gi * P:(gi + 1) * P],
                                            ident[:])
                        nc.vector.tensor_copy(stsbig[:, g * P:(g + 1) * P], stp[:])
                    mm(qps[:, j * 8:(j + 1) * 8], lhsT=stsbig[:, g * P:(g + 1) * P],
                       rhs=QOH[g], start=True, stop=True)
                qcat = wp.tile([P, P], f16, tag="qcat", name="qcat")
                nc.vector.tensor_copy(qcat[:, 0:jn * 8], qps[:, 0:jn * 8])
                qv = qcat[:, 0:jn * 8].rearrange("p (g t f) -> p g t f", t=2, f=4)
                ncols = jn * 4
                hps = pM.tile([P, ncols], f32, tag="msg", name="hps",
                              padded_shape=[P, 256])
                mm(hps[:], lhsT=cw1[:, 0:P], rhs=qv[:, :, 0, :], start=True, stop=False)
                mm(hps[:], lhsT=cw1[:, P:256], rhs=qv[:, :, 1, :], start=False, stop=True)
                hg = wp.tile([P, ncols], f16, tag="hg", name="hg",
                             padded_shape=[P, 64])
                nc.scalar.activation(hg[:], hps[:], AF.Gelu, bias=cb1[:, 0:1])
                ops_ = pG.tile([P, ncols], f32, tag="agg", name="ops",
                               padded_shape=[P, 256])
                mm(ops_[:], lhsT=cw2[:], rhs=hg[:], start=True, stop=True)
                nc.scalar.activation(
                    outsb[:, bq * 64:bq * 64 + ncols], ops_[0:20, :], AF.Identity,
                    bias=cb2[:, 0:1])
            nc.sync.dma_start(d_out[:], outsb[:])

    nc.finalize()
    return nc


def _pack_samples(ne, b_core):
    """Global balanced assignment: returns per-core list of
    (group, j) -> original sample index arrays, shape (n_cores, G, 4)."""
    B = ne.shape[0]
    n_cores = B // b_core
    G = b_core // GRP
    caps = _caps_for(b_core)
    n_thin = sum(1 for c in caps if c == 128)
    nfat = G - n_thin

    order = np.argsort(-ne, kind="stable")
    core_sum = np.zeros(n_cores, dtype=np.int64)
    core_cnt = np.zeros(n_cores, dtype=np.int64)
    core_members = [[] for _ in range(n_cores)]
    for s in order:
        avail = np.where(core_cnt < b_core)[0]
        c = avail[np.argmin(core_sum[avail])]
        core_members[c].append(s)
        core_sum[c] += ne[s]
        core_cnt[c] += 1

    layout = np.zeros((n_cores, G, GRP), dtype=np.int64)
    ok = True
    for c in range(n_cores):
        mem = np.asarray(core_members[c])
        o = np.argsort(-ne[mem], kind="stable")
        mem = mem[o]
        fat = mem[:4 * nfat]
        rest = mem[4 * nfat:]
        # fat groups occupy the END of the group list (indices n_thin..G-1)
        for fg in range(nfat):
            layout[c, n_thin + fg] = fat[fg * 4:(fg + 1) * 4]
        nr = ne[rest]
        sums = np.zeros(n_thin, dtype=np.int64)
        cnt = np.zeros(n_thin, dtype=np.int64)
        for k in range(rest.shape[0]):
            avail = np.where(cnt < GRP)[0]
            gsel = avail[np.argmin(sums[avail])]
            layout[c, gsel, cnt[gsel]] = rest[k]
            sums[gsel] += nr[k]
            cnt[gsel] += 1
        if n_thin and sums.max() > 128:
            ok = False
    return layout, ok


def _host_prep(inputs, b_core):
    f = np.float32
    h = np.float16
    src_a = np.asarray(inputs["edge_src"]).astype(np.int64)
    tgt_a = np.asarray(inputs["edge_tgt"]).astype(np.int64)
    rel_a = np.asarray(inputs["edge_rel"]).astype(np.int64)
    ne = np.asarray(inputs["n_edges"]).astype(np.int64)
    qs_a = np.asarray(inputs["query_src"]).astype(np.int64)
    qt_a = np.asarray(inputs["query_tgt"]).astype(np.int64)
    B = src_a.shape[0]
    n_cores = B // b_core
    G = b_core // GRP
    NPAIR = G // 2
    caps = _caps_for(b_core)
    n_thin = sum(1 for c in caps if c == 128)
    nfat = G - n_thin
    n_reltile = (n_thin + 3) // 4

    layout, ok = _pack_samples(ne, b_core)
    if not ok:
        # fall back: no thin groups (callers rebuild nc keyed by caps anyway)
        raise RuntimeError("edge packing overflow; unsupported input")

    # shared params
    ee = np.asarray(inputs["entity_embed"], f)
    w1 = np.asarray(inputs["msg_W1"], f)
    reltab = np.asarray(inputs["rel_embed"], f) @ w1[128:256] + np.asarray(inputs["msg_b1"], f)
    # band layout: rows 0:20 hold the F0 half of the rel table, rows 32:52 the
    # F1 half; the rel one-hot rhs selects the right band per column range
    rt2 = np.zeros((P, 128), f)
    rt2[0:20] = reltab[:, 0:128]
    rt2[32:52] = reltab[:, 128:256]
    w2m_ = np.asarray(inputs["msg_W2"], f)
    w1u_ = np.asarray(inputs["upd_W1"], f)
    w2u_ = np.asarray(inputs["upd_W2"], f)
    cw1_ = np.asarray(inputs["cls_W1"], f)
    cw2p = np.zeros((P, 128), f)
    cw2p[:, 0:20] = np.asarray(inputs["cls_W2"], f)
    shared = {
        "s0": np.ascontiguousarray(ee.T.astype(h)),
        "rt2": rt2.astype(h),
        "b2m": np.asarray(inputs["msg_b2"], f).reshape(128, 1).copy(),
        "w1ac": np.concatenate([w1[0:128], w1[256:384]], axis=1).astype(h),
        "w2m": np.concatenate([w2m_[0:128], w2m_[128:256]], axis=1).astype(h),
        "w1u": np.concatenate(
            [w1u_[0:128, 0:128], w1u_[0:128, 128:256],
             w1u_[128:256, 0:128], w1u_[128:256, 128:256]], axis=1).astype(h),
        "w2u": np.concatenate([w2u_[0:128], w2u_[128:256]], axis=1).astype(h),
        "b1u": np.asarray(inputs["upd_b1"], f).reshape(2, 128).T.copy(),
        "b2u": np.asarray(inputs["upd_b2"], f).reshape(128, 1).copy(),
        "cw1": np.concatenate([cw1_[0:128], cw1_[128:256]], axis=1).astype(h),
        "cb1": np.asarray(inputs["cls_b1"], f).reshape(128, 1).copy(),
        "cw2": cw2p.astype(h),
        "cb2": np.asarray(inputs["cls_b2"], f).reshape(20, 1).copy(),
    }

    n_thinp = n_thin // 2
    n_fatp = nfat // 2
    in_maps = []
    perm = np.zeros(B, dtype=np.int64)  # out row (c*b_core + g*4 + j) -> sample
    for c in range(n_cores):
        bund_t = np.zeros((max(1, n_thinp), P, 1280), h)
        bund_f = np.zeros((max(1, n_fatp), P, 2560), h)
        ind = np.zeros((NPAIR, 256), np.float32)
        qoh = np.zeros((P, G * 8), h)

        thin_p = 0
        fat_p = 0
        for p in range(NPAIR):
            fatpair = caps[2 * p] == 256
            if fatpair:
                bund = bund_f[fat_p]
                fat_p += 1
            else:
                bund = bund_t[thin_p]
                thin_p += 1
            cap = caps[2 * p]
            cw = cap
            for gi in (0, 1):
                g = 2 * p + gi
                rtile = bund[:, gi * 2 * cw:(gi + 1) * 2 * cw]
                ohs = bund[:, 4 * cw + gi * cw:4 * cw + (gi + 1) * cw]
                oht = bund[:, 6 * cw + gi * cw:6 * cw + (gi + 1) * cw]
                ohe = bund[:, 8 * cw + gi * cw:8 * cw + (gi + 1) * cw]
                pos = 0
                for j in range(GRP):
                    s = layout[c, g, j]
                    perm[c * b_core + g * GRP + j] = s
                    nedge = ne[s]
                    for e in range(nedge):
                        ohs[j * 32 + src_a[s, e], pos] = 1
                        oht[j * 32 + tgt_a[s, e], pos] = 1
                        ec, pr = divmod(pos, 128)
                        ohe[pr, ec * 128 + j * 32 + tgt_a[s, e]] = 1
                        rtile[rel_a[s, e], pos] = 1
                        rtile[32 + rel_a[s, e], cap + pos] = 1
                        ind[p, gi * 128 + j * 32 + tgt_a[s, e]] += 1
                        pos += 1
                    assert pos <= cap
                    qoh[j * 32 + qs_a[s], g * 8 + j] = 1
                    qoh[j * 32 + qt_a[s], g * 8 + 4 + j] = 1

        indb = np.ascontiguousarray(
            np.broadcast_to(ind[:, None, :], (NPAIR, P, 256))
            .reshape(NPAIR // 8, 8, P, 256).transpose(0, 2, 1, 3)
            .reshape(NPAIR // 8, P, 8 * 256)).astype(h)
        m = dict(shared)
        if n_thinp:
            m["bund_t"] = bund_t
        if n_fatp:
            m["bund_f"] = bund_f
        m.update({"indb": indb, "qoh": qoh})
        in_maps.append(m)
    return in_maps, perm


_CACHE = {}


def _make_in_maps(inputs, b_core):
    return _host_prep(inputs, b_core)[0]


def kernel(**inputs):
    b = np.asarray(inputs["edge_src"]).shape[0]
    b_core = b // N_CORES
    _patch_ldw_opt()
    if b_core not in _CACHE:
        _CACHE[b_core] = _build_nc(b_core, N_STEPS)
    nc = _CACHE[b_core]

    in_maps, perm = _host_prep(inputs, b_core)

    from concourse.bass_utils import run_bass_kernel_spmd
    res = run_bass_kernel_spmd(nc, in_maps, core_ids=list(range(N_CORES)))
    packed = np.concatenate([r["out"].T for r in res.results], axis=0)
    out = np.zeros_like(packed)
    out[perm] = packed
    return np.ascontiguousarray(out, dtype=np.float32)
